# revision 110
# baseline (speedup 1.0000x reference)
"""Trainium2 Bass kernel for nn_Block_45724221833665 (dense transformer block).

Strategy: pure data-parallel over batch — 8 batch elements, 8 NeuronCores, one
batch element per core, no collectives.  Feature-major compute throughout.

v2 rewrite, engineered around the PE p-state ramp (the tensor engine runs at
1.2 GHz until it has been continuously busy for 3 us, 2.4 GHz after): the
emission order is arranged so the PE never stalls.

  * LN1/LN3 are DEFERRED into the Q/K/V projections: the matmuls run on the
    raw transposed input (bf16), and the normalization becomes a fused psum
    epilogue  out = s[t] * (P + (-m[t]) * wsum[d])  on DVE
    (scalar_tensor_tensor + tensor_tensor).  The key-side rstd is folded into
    the EXP activation's per-partition scale AP, so kT needs no epilogue
    multiply at all.  The PE starts projecting immediately after the
    transposes; the LN row computation overlaps the first matmuls.
  * The softmax denominators use DVE reciprocal_approx_fast instead of the
    ACT-table reciprocal: ACT keeps the exp table loaded for the whole
    attention phase (no table thrash) and stays exp-only there (it is the
    attention-phase bottleneck at ~18us/pair).
  * Q/K projections for head pairs 2..7 and the second half of V are
    interleaved INTO the attention pair loop, filling the PE while ACT chews
    the exp stream.
  * LN2 and the ffn_ln run split by token halves: stats/rows/apply for tokens
    0-511 overlap the second half's matmuls (proj s1, fc1 s1), so the row
    latency never idles the PE.  subln and ffn_ln stay deferred into the
    following matmul (rank-1 mean correction + rstd scale in the epilogue).
  * The residual uses xbT (bf16 x^T) directly — no f32 x bounce to DRAM; the
    post-attention state xn is kept in bf16 only.
  * W1 is resident in SBUF (DMA'd during proj, after attention frees SBUF);
    W1/W2 are each streamed from HBM exactly once.
"""

import numpy as np
import ml_dtypes

import concourse.bass as bass
import concourse.mybir as mybir
import concourse.tile as tile
from concourse import bacc

B, N, PT, D, H, HD, HID = 8, 1024, 64, 1024, 16, 64, 4096
KT = D // 128          # 8 contraction tiles over D
HT = HID // 128        # 32 tiles over HID
DT = D // 128          # 8 output tiles over D
NTOK = N               # 1024 main tokens per core
NKEY = N + PT          # 1088 keys (main tokens + text prefix)
HLF = NTOK // 2        # 512 token half
EPS = 1e-5

F32 = mybir.dt.float32
BF16 = mybir.dt.bfloat16
FP8 = mybir.dt.float8e4
AF = mybir.ActivationFunctionType
ALU = mybir.AluOpType
DROW = mybir.MatmulPerfMode.DoubleRow


def build_program(debug=(), triv=frozenset()):
    nc = bacc.Bacc("TRN2")
    dbg = set(debug)
    triv = set(triv)

    # ---- I/O ------------------------------------------------------------
    x_d = nc.declare_dram_parameter("x", [NTOK, D], F32, isOutput=False)
    xt_d = nc.declare_dram_parameter("x_text", [PT, D], F32, isOutput=False)
    # Q/K weights blocked by output tile m; V blocked by output half sn.
    # Q/K run in fp8 DoubleRow (host-prescaled by 16 out of e4m3's denormal
    # range; the 1/16 is folded into the s8 row / the exp scale row) — their
    # elementwise errors average out in the softmax.
    wq_d = nc.declare_dram_parameter("wq", [DT, 128, KT, 128], FP8,
                                     isOutput=False)
    wk_d = nc.declare_dram_parameter("wk", [DT, 128, KT, 128], FP8,
                                     isOutput=False)
    wv_d = nc.declare_dram_parameter("wv", [2, 128, KT, 512], BF16,
                                     isOutput=False)
    wp_d = nc.declare_dram_parameter("wp", [KT, 128, D], BF16, isOutput=False)
    w1_d = nc.declare_dram_parameter("w1", [HT, 128, KT, 128], BF16,
                                     isOutput=False)
    w2_d = nc.declare_dram_parameter("w2", [DT, 128, HT, 128], BF16,
                                     isOutput=False)
    pvec = {}
    for nm, sz in [
        ("bq8", D), ("bp", D), ("b2", D), ("kb", D), ("bv_eff", D),
        ("b1", HID),
        ("wqsum", D), ("wksum", D), ("wvsum", D), ("wpsum", D), ("w2sum", D),
    ]:
        pvec[nm] = nc.declare_dram_parameter(nm, [sz], F32, isOutput=False)
    ident_d = nc.declare_dram_parameter("ident", [128, 128], F32,
                                        isOutput=False)
    gate_d = nc.declare_dram_parameter("gate", [H], F32, isOutput=False)
    y_d = nc.declare_dram_parameter("y", [NTOK, D], F32, isOutput=True)

    dbg_d = {}

    def dbg_out(name, shape, dtype):
        if name in dbg:
            dbg_d[name] = nc.declare_dram_parameter(
                "dbg_" + name, list(shape), dtype, isOutput=True
            )

    dbg_out("xbT", [128, DT, NTOK], BF16)
    dbg_out("qT", [128, DT, NTOK], BF16)
    dbg_out("kT", [128, DT, NKEY], BF16)
    dbg_out("v", [128, KT, H, HD + 1], FP8)
    dbg_out("v_text", [PT, H, HD + 1], FP8)
    dbg_out("c_vec", [D], F32)
    dbg_out("o_lnT", [128, DT, NTOK], BF16)
    dbg_out("pp00", [128, KT, NTOK], BF16)
    dbg_out("se0", [2, NTOK], F32)
    dbg_out("rb0", [128, NTOK], F32)
    dbg_out("opr0", [128, NTOK], F32)
    dbg_out("xnbT", [128, DT, NTOK], BF16)
    dbg_out("x2T", [128, DT, NTOK], BF16)
    dbg_out("hT", [128, HT, NTOK], BF16)

    with tile.TileContext(nc) as tc:
        _build(nc, tc, x_d, xt_d, wq_d, wk_d, wv_d, wp_d,
               w1_d, w2_d, pvec, gate_d, y_d, dbg_d, triv, ident_d)
    nc.compile()
    return nc


def _build(nc, tc, x_d, xt_d, wq_d, wk_d, wv_d, wp_d, w1_d, w2_d,
           pvec, gate_d, y_d, dbg_d, triv, ident_d):
    import contextlib
    ctx = contextlib.ExitStack()
    consts = ctx.enter_context(tc.tile_pool(name="consts", bufs=1))
    rows = ctx.enter_context(tc.tile_pool(name="rows", bufs=1))
    tmps = ctx.enter_context(tc.tile_pool(name="tmps", bufs=3))
    sqp = ctx.enter_context(tc.tile_pool(name="sqp", bufs=2))
    rwp = ctx.enter_context(tc.tile_pool(name="rwp", bufs=2))
    dram = ctx.enter_context(tc.tile_pool(name="dram", bufs=1, space="DRAM"))
    ps_mm = ctx.enter_context(tc.tile_pool(name="ps_mm", bufs=2, space="PSUM"))
    ps_sc = ctx.enter_context(tc.tile_pool(name="ps_sc", bufs=3, space="PSUM"))

    def mm_psum(pfree=512, parts=128):
        t = ps_mm.tile([128, 512], F32, tag="mm", name="mmps")
        return t[:parts, :pfree]

    # ---- constants / parameter DMAs ------------------------------------
    ident = consts.tile([128, 128], F32, name="ident")
    nc.sync.dma_start(out=ident, in_=ident_d[:, :])
    ones_b = consts.tile([128, 1], BF16, name="ones_b")
    nc.vector.memset(ones_b, 1.0)
    ones2 = consts.tile([128, 64], BF16, name="ones2")
    nc.vector.memset(ones2, 1.0)
    eps_c = consts.tile([64, 1], F32, name="eps_c")
    nc.vector.memset(eps_c, EPS)

    st = {}
    for nm in ["bq8", "bp", "b2", "kb", "wqsum", "wksum", "wpsum", "w2sum"]:
        t = consts.tile([128, DT], F32, name="st_" + nm)
        nc.sync.dma_start(out=t, in_=pvec[nm].rearrange("(o p) -> p o", p=128))
        st[nm] = t
    t = consts.tile([128, HT], F32, name="st_b1")
    nc.sync.dma_start(out=t, in_=pvec["b1"].rearrange("(o p) -> p o", p=128))
    st["b1"] = t

    # xbT sits at the bottom of the left stack (lives until the proj
    # residual); everything allocated above it is released in LIFO order.
    p_xbT = tc.alloc_tile_pool(name="p_xbT", bufs=1)
    xbT = p_xbT.tile([128, DT, NTOK], BF16, name="xbT")
    p_xtb = tc.alloc_tile_pool(name="p_xtb", bufs=1)
    xtb = p_xtb.tile([128, DT, PT], BF16, name="xtb")
    # fp8 copies of x^T / x_text^T for the fp8 Q/K projections
    xb8 = p_xtb.tile([128, DT, NTOK], FP8, name="xb8")
    xtb8 = p_xtb.tile([128, DT, PT], FP8, name="xtb8")

    # early broadcast pool (released after the QKV projections)
    ebc = tc.alloc_tile_pool(name="ebc", bufs=1)
    wvsum_b = ebc.tile([128, D], F32, name="wvsum_b")
    nc.sync.dma_start(
        out=wvsum_b,
        in_=pvec["wvsum"].rearrange("(a d) -> a d", a=1).to_broadcast((128, D)))
    negm_b = ebc.tile([128, NTOK], F32, name="negm_b")
    s8_b = ebc.tile([128, NTOK], F32, name="s8_b")
    negm3_b = ebc.tile([128, PT], F32, name="negm3_b")
    # small striped columns (alive through attention)
    colp = tc.alloc_tile_pool(name="colp", bufs=1)
    s_col = colp.tile([128, KT], F32, name="s_col")
    s16_col = colp.tile([128, KT], F32, name="s16_col")
    nm_col = colp.tile([128, KT], F32, name="nm_col")
    s3_col = colp.tile([PT, 1], F32, name="s3_col")
    nm3_col = colp.tile([PT, 1], F32, name="nm3_col")
    s3_col2 = colp.tile([128, 1], F32, name="s3_col2")

    # ---- LN row machinery -----------------------------------------------
    # Row math runs in [64, w//64] layout (engine APs must start at
    # partition 0/64, and a [1, w] tile reserves w*4 bytes on EVERY
    # partition — the 2D layout costs next to nothing).  PSUM stat rows are
    # staged through a [1, 2*HLF] tile, bounced to DRAM (DMA reshapes
    # freely), mathed, and the result rows land in a [5, w] DRAM tile:
    # row 0=mean, 1=-mean, 2=rstd, 3=rstd/8, 4=-mean*rstd.
    stage = rows.tile([1, 2 * HLF], F32, name="stage")

    def ln_chain(ps_a, ps_b, w, n_elems, name, s_scale=None):
        """ps_a/ps_b: [1, w] APs (psum or sbuf rows) with sum / sum-of-sq.
        Returns a [5, w] DRAM tile (mean, negm, s, s8, ns rows)."""
        nc.vector.tensor_copy(out=stage[:, 0:w], in_=ps_a)
        nc.vector.tensor_copy(out=stage[:, w:2 * w], in_=ps_b)
        bin_ = dram.tile([2, NTOK], F32, tag="lnbin", bufs=2, name=name + "i")
        nc.sync.dma_start(out=bin_[0:1, :w], in_=stage[:, 0:w])
        nc.sync.dma_start(out=bin_[1:2, :w], in_=stage[:, w:2 * w])
        return ln_math(bin_, w, n_elems, name, s_scale)

    def ln_math(bin_, w, n_elems, name, s_scale=None):
        wf = w // 64
        r = rwp.tile([64, 9, 16], F32, tag="rw", name=name + "r")
        r_sum, r_sq, r_t, r_u = (r[:, i, :wf] for i in range(4))
        r_negm, r_s, r_s8, r_ns = (r[:, i, :wf] for i in range(4, 8))
        r_s16 = r[:, 8, :wf]
        nc.sync.dma_start(
            out=r_sum, in_=bin_[0:1, :w].rearrange("a (p f) -> (a p) f", p=64))
        nc.sync.dma_start(
            out=r_sq, in_=bin_[1:2, :w].rearrange("a (p f) -> (a p) f", p=64))
        inv = 1.0 / float(n_elems)
        nc.vector.tensor_scalar_mul(out=r_sum, in0=r_sum, scalar1=inv)
        nc.vector.tensor_scalar_mul(out=r_sq, in0=r_sq, scalar1=inv)
        nc.vector.tensor_tensor(r_t, r_sum, r_sum, ALU.mult)
        nc.vector.tensor_tensor(r_sq, r_sq, r_t, ALU.subtract)  # var
        nc.scalar.activation(out=r_s, in_=r_sq, func=AF.Abs_reciprocal_sqrt,
                             bias=eps_c, scale=1.0)
        # rsqrt Newton: s <- s*(1.5 - 0.5*(var+eps)*s^2)
        nc.vector.tensor_scalar_add(out=r_t, in0=r_sq, scalar1=EPS)
        nc.vector.tensor_tensor(r_u, r_s, r_s, ALU.mult)
        nc.vector.tensor_tensor(r_u, r_u, r_t, ALU.mult)
        nc.vector.tensor_scalar(out=r_u, in0=r_u, scalar1=-0.5,
                                scalar2=1.5, op0=ALU.mult, op1=ALU.add)
        nc.vector.tensor_tensor(r_s, r_s, r_u, ALU.mult)
        nc.vector.tensor_scalar_mul(out=r_negm, in0=r_sum, scalar1=-1.0)
        # s8 row absorbs the 1/16 un-scaling of the fp8 Wq (0.125/16)
        nc.vector.tensor_scalar_mul(out=r_s8, in0=r_s, scalar1=0.0078125)
        nc.vector.tensor_tensor(r_ns, r_negm, r_s, ALU.mult)
        # s16 row: rstd/16 — the exp scale un-scaling the fp8 Wk
        nc.vector.tensor_scalar_mul(out=r_s16, in0=r_s, scalar1=0.0625)
        bout = dram.tile([6, NTOK], F32, tag="lnbout", bufs=2,
                         name=name + "o")
        for i, src in enumerate([r_sum, r_negm, r_s, r_s8, r_ns, r_s16]):
            nc.sync.dma_start(
                out=bout[i:i + 1, :w].rearrange("a (p f) -> (a p) f", p=64),
                in_=src)
        return bout

    # =====================================================================
    # PH1: load x / x_text, transpose -> xbT (bf16, raw).  LN1 stats/rows
    # per token half; LN3 for text.
    # =====================================================================
    def emit_ln1_half(h, ps_a, ps_b):
        sl = slice(h * HLF, (h + 1) * HLF)
        bout = ln_chain(ps_a, ps_b, HLF, D, f"ln1_{h}")
        nc.sync.dma_start(out=negm_b[:, sl],
                          in_=bout[1:2, :HLF].to_broadcast((128, HLF)))
        nc.sync.dma_start(out=s8_b[:, sl],
                          in_=bout[3:4, :HLF].to_broadcast((128, HLF)))
        nc.sync.dma_start(
            out=s_col[:, h * 4:(h + 1) * 4],
            in_=bout[2:3, :HLF].rearrange("a (o p) -> (a p) o", p=128))
        nc.sync.dma_start(
            out=s16_col[:, h * 4:(h + 1) * 4],
            in_=bout[5:6, :HLF].rearrange("a (o p) -> (a p) o", p=128))
        nc.sync.dma_start(
            out=nm_col[:, h * 4:(h + 1) * 4],
            in_=bout[1:2, :HLF].rearrange("a (o p) -> (a p) o", p=128))

    with tc.tile_pool(name="p_x", bufs=1) as p_x:
        x_sb = p_x.tile([128, DT, D], F32, name="x_sb")
        for hf in range(4):
            nc.sync.dma_start(
                out=x_sb[:, hf * 2:(hf + 1) * 2, :],
                in_=x_d[hf * 256:(hf + 1) * 256, :].rearrange(
                    "(t p) d -> p t d", p=128))
        xt_sb = p_x.tile([PT, D], F32, name="xt_sb")
        nc.sync.dma_start(out=xt_sb, in_=xt_d[:, :])

        def stat_ps():
            t = ps_sc.tile([128, 1024], F32, tag="sc", name="statps")
            return t[:1, 0:512], t[:1, 512:1024]

        for h in range(2):  # token halves
            sl = slice(h * HLF, (h + 1) * HLF)
            for t in range(h * 4, (h + 1) * 4):
                for o in range(DT):
                    pst = mm_psum(128)
                    nc.tensor.transpose(
                        pst, x_sb[:, t, o * 128:(o + 1) * 128], ident)
                    nc.vector.tensor_copy(
                        out=xbT[:, o, t * 128:(t + 1) * 128], in_=pst)
                    nc.scalar.copy(
                        out=xb8[:, o, t * 128:(t + 1) * 128], in_=pst)
            ps_a, ps_b = stat_ps()
            for o in range(DT):
                sq_t = sqp.tile([128, NTOK], BF16, tag="sq_t",
                                name="sq_t")[:, :HLF]
                nc.vector.tensor_tensor(sq_t, xbT[:, o, sl], xbT[:, o, sl],
                                        ALU.mult)
                nc.tensor.matmul(ps_a, ones_b, xbT[:, o, sl],
                                 start=(o == 0), stop=(o == DT - 1),
                                 skip_group_check=True)
                nc.tensor.matmul(ps_b, ones_b, sq_t,
                                 start=(o == 0), stop=(o == DT - 1),
                                 skip_group_check=True)
            emit_ln1_half(h, ps_a, ps_b)

        # ---- text: transpose + LN3 rows --------------------------------
        for o in range(DT):
            pst = mm_psum(PT)
            nc.tensor.transpose(
                pst, xt_sb[:, o * 128:(o + 1) * 128], ident[:PT, :PT])
            nc.vector.tensor_copy(out=xtb[:, o, :], in_=pst)
            nc.scalar.copy(out=xtb8[:, o, :], in_=pst)
        ps_a, ps_b = stat_ps()
        ps_a = ps_a[:, :PT]
        ps_b = ps_b[:, :PT]
        for o in range(DT):
            sq_t = sqp.tile([128, NTOK], BF16, tag="sq_t",
                            name="sq_t3")[:, :PT]
            nc.vector.tensor_tensor(sq_t, xtb[:, o, :], xtb[:, o, :],
                                    ALU.mult)
            nc.tensor.matmul(ps_a, ones_b, xtb[:, o, :],
                             start=(o == 0), stop=(o == DT - 1),
                             skip_group_check=True)
            nc.tensor.matmul(ps_b, ones_b, sq_t,
                             start=(o == 0), stop=(o == DT - 1),
                             skip_group_check=True)
        bout3 = ln_chain(ps_a, ps_b, PT, D, "ln3")
        nc.sync.dma_start(out=negm3_b,
                          in_=bout3[1:2, :PT].to_broadcast((128, PT)))
        nc.sync.dma_start(
            out=s3_col,
            in_=bout3[2:3, :PT].rearrange("a (o p) -> (a p) o", p=PT))
        nc.sync.dma_start(
            out=nm3_col,
            in_=bout3[1:2, :PT].rearrange("a (o p) -> (a p) o", p=PT))
        nc.sync.dma_start(
            out=s3_col2[0:PT],
            in_=bout3[5:6, :PT].rearrange("a (o p) -> (a p) o", p=PT))
        nc.sync.dma_start(
            out=s3_col2[PT:128],
            in_=bout3[5:6, :PT].rearrange("a (o p) -> (a p) o", p=PT))

    if "xbT" in dbg_d:
        nc.sync.dma_start(out=dbg_d["xbT"][:], in_=xbT[:])

    # =====================================================================
    # Projections (deferred-LN epilogues) + attention, interleaved.
    # =====================================================================
    p_qkv = tc.alloc_tile_pool(name="p_qkv", bufs=1)
    qT = p_qkv.tile([128, DT, NTOK], BF16, name="qT")
    kT = p_qkv.tile([128, DT, NKEY], BF16, name="kT")
    v_sb = p_qkv.tile([128, KT, H, HD + 1], FP8, name="v_sb")
    vt_sb = p_qkv.tile([PT, H, HD + 1], FP8, name="vt_sb")
    # text V duplicated at partition bases 0 and 64 so the av matmul's
    # stationary base matches the packed ppt moving operand
    vt2 = p_qkv.tile([128, H, HD + 1], FP8, name="vt2")

    p_wblk = tc.alloc_tile_pool(name="p_wblk", bufs=4)
    p_wvh = tc.alloc_tile_pool(name="p_wvh", bufs=2)

    skip_bq = "bq" in triv
    skip_kb = "kb" in triv

    def emit_q(m):
        blk = p_wblk.tile([128, KT, 128], FP8, tag="wblk", name="wqblk")
        nc.sync.dma_start(out=blk, in_=wq_d[m])
        for s in range(2):
            sl = slice(s * HLF, (s + 1) * HLF)
            ps = mm_psum()
            for o in range(KT // 2):
                nc.tensor.matmul(
                    ps, blk[:, 2 * o:2 * o + 2, :],
                    xb8[:, 2 * o:2 * o + 2, sl],
                    start=(o == 0), stop=(o == KT // 2 - 1), perf_mode=DROW)
            tmp = tmps.tile([128, 512], F32, tag="wrk", name="q_tmp")
            nc.vector.scalar_tensor_tensor(
                out=tmp, in0=negm_b[:, sl], scalar=st["wqsum"][:, m:m + 1],
                in1=ps, op0=ALU.mult, op1=ALU.add)
            nc.vector.tensor_tensor(qT[:, m, sl], tmp, s8_b[:, sl], ALU.mult)
            if not skip_bq:
                nc.vector.tensor_scalar_add(
                    out=qT[:, m, sl], in0=qT[:, m, sl],
                    scalar1=st["bq8"][:, m:m + 1])

    def emit_k(m):
        blk = p_wblk.tile([128, KT, 128], FP8, tag="wblk", name="wkblk")
        nc.sync.dma_start(out=blk, in_=wk_d[m])
        for s in range(2):
            sl = slice(s * HLF, (s + 1) * HLF)
            ps = mm_psum()
            for o in range(KT // 2):
                nc.tensor.matmul(
                    ps, blk[:, 2 * o:2 * o + 2, :],
                    xb8[:, 2 * o:2 * o + 2, sl],
                    start=(o == 0), stop=(o == KT // 2 - 1), perf_mode=DROW)
            nc.vector.scalar_tensor_tensor(
                out=kT[:, m, sl], in0=negm_b[:, sl],
                scalar=st["wksum"][:, m:m + 1], in1=ps,
                op0=ALU.mult, op1=ALU.add)
            if not skip_kb:
                nc.vector.tensor_scalar_add(
                    out=kT[:, m, sl], in0=kT[:, m, sl],
                    scalar1=st["kb"][:, m:m + 1])
        ps = mm_psum(PT)
        for o in range(KT // 2):
            nc.tensor.matmul(
                ps, blk[:, 2 * o:2 * o + 2, :],
                xtb8[:, 2 * o:2 * o + 2, :],
                start=(o == 0), stop=(o == KT // 2 - 1), perf_mode=DROW)
        nc.vector.scalar_tensor_tensor(
            out=kT[:, m, N:N + PT], in0=negm3_b,
            scalar=st["wksum"][:, m:m + 1], in1=ps,
            op0=ALU.mult, op1=ALU.add)
        if not skip_kb:
            nc.vector.tensor_scalar_add(
                out=kT[:, m, N:N + PT], in0=kT[:, m, N:N + PT],
                scalar1=st["kb"][:, m:m + 1])

    def load_wv(sn):
        wvh = p_wvh.tile([128, KT, 512], BF16, tag="wvh", name="wvh")
        nc.sync.dma_start(out=wvh, in_=wv_d[sn])
        return wvh

    def emit_v(wvh, sn, t):
        ps = mm_psum()
        for o in range(KT):
            nc.tensor.matmul(
                ps, xbT[:, o, t * 128:(t + 1) * 128], wvh[:, o, :],
                start=(o == 0), stop=(o == KT - 1))
        tmp = tmps.tile([128, 512], F32, tag="wrk", name="v_tmp")
        sl = slice(sn * 512, (sn + 1) * 512)
        nc.vector.scalar_tensor_tensor(
            out=tmp, in0=wvsum_b[:, sl], scalar=nm_col[:, t:t + 1],
            in1=ps, op0=ALU.mult, op1=ALU.add)
        out = v_sb[:, t, sn * 8:(sn + 1) * 8, 0:HD]
        nc.vector.tensor_scalar_mul(out=out, in0=tmp,
                                    scalar1=s_col[:, t:t + 1])

    # f32 copy of v_text row 0 (c_vec must not eat the fp8 quantization —
    # its error would bias every token)
    v0_row = rows.tile([1, D], F32, name="v0_row")

    def emit_vt(wvh, sn):
        ps = mm_psum(parts=PT)
        for o in range(KT):
            nc.tensor.matmul(
                ps, xtb[:, o, :], wvh[:, o, :],
                start=(o == 0), stop=(o == KT - 1))
        tmp = tmps.tile([128, 512], F32, tag="wrk", name="vt_tmp")[:PT, :]
        sl = slice(sn * 512, (sn + 1) * 512)
        nc.vector.scalar_tensor_tensor(
            out=tmp, in0=wvsum_b[:PT, sl], scalar=nm3_col,
            in1=ps, op0=ALU.mult, op1=ALU.add)
        out = vt_sb[:, sn * 8:(sn + 1) * 8, 0:HD]
        nc.vector.tensor_scalar_mul(out=out, in0=tmp, scalar1=s3_col)
        nc.vector.tensor_scalar_mul(out=v0_row[:, sl], in0=tmp[0:1, :],
                                    scalar1=s3_col[0:1])

    # pre-attention: only what pair 0's scores need (Q0, K0, Q1, K1) — all
    # V work moves into the first filler slots so the ACT exp stream starts
    # as early as possible
    emit_q(0)
    emit_k(0)
    emit_q(1)
    emit_k(1)
    wvh0 = load_wv(0)
    wvh1 = load_wv(1)

    # ---- c_vec: tanh(gate)*v0_raw + (1+tanh(gate))*bv_eff ---------------
    grws = rows.tile([1, 3, H], F32, name="grws")
    cvw = rows.tile([128, H, HD], F32, name="cvw")
    c_st = consts.tile([128, DT], F32, name="c_st")

    def emit_cvec():
        g_row = grws[:, 0, :]
        th_row = grws[:, 1, :]
        c_work = cvw[0:1]
        nc.sync.dma_start(out=g_row,
                          in_=gate_d.rearrange("(a h) -> a h", a=1))
        nc.scalar.activation(out=th_row, in_=g_row, func=AF.Tanh)
        nc.vector.tensor_copy(
            out=c_work, in_=v0_row.rearrange("a (h d) -> a h d", h=H))
        nc.vector.tensor_tensor(
            c_work, c_work, th_row[:, :, None].to_broadcast((1, H, HD)),
            ALU.mult)
        if "bv" not in triv:
            th1_row = grws[:, 2, :]
            nc.scalar.activation(out=th1_row, in_=th_row, func=AF.Identity,
                                 bias=1.0)
            bv_row = cvw[64:65]
            nc.sync.dma_start(
                out=bv_row,
                in_=pvec["bv_eff"].rearrange("(a h d) -> a h d", a=1, h=H))
            nc.vector.tensor_tensor(
                bv_row, bv_row, th1_row[:, :, None].to_broadcast((1, H, HD)),
                ALU.mult)
            nc.vector.tensor_tensor(c_work, c_work, bv_row, ALU.add)
        c_dram = dram.tile([D], F32, name="c_dram")
        nc.sync.dma_start(
            out=c_dram.rearrange("(a h d) -> a h d", a=1, h=H), in_=c_work)
        nc.sync.dma_start(out=c_st,
                          in_=c_dram.rearrange("(o p) -> p o", p=128))
        if "c_vec" in dbg_d:
            nc.sync.dma_start(out=dbg_d["c_vec"][:], in_=c_dram[:])
        # ones column + zero out reference-key-0 (first text token)
        nc.vector.memset(v_sb[:, :, :, HD:HD + 1], 1.0)
        nc.vector.memset(vt_sb[:, :, HD:HD + 1], 1.0)
        nc.vector.memset(vt_sb[0:1, :, :], 0.0)
        nc.vector.tensor_copy(out=vt2[0:PT], in_=vt_sb)
        nc.vector.tensor_copy(out=vt2[64:64 + PT], in_=vt_sb)

    # =====================================================================
    # Attention: 8 head pairs; filler QKV work interleaved per slot.
    # =====================================================================
    p_OlnT = tc.alloc_tile_pool(name="p_OlnT", bufs=1, side="right")
    o_lnT = p_OlnT.tile([128, DT, NTOK], BF16, name="o_lnT")
    # subln stat accumulators: rows at partitions 0 / 64 of one 4KB tile
    oacc = rows.tile([128, NTOK], F32, name="oacc")
    sumO_row = oacc[0:1, :]
    sqO_row = oacc[64:65, :]
    nc.vector.memset(sumO_row, 0.0)
    nc.vector.memset(sqO_row, 0.0)

    # filler work sits as LATE as dependencies allow: the ACT exp stream is
    # the attention bottleneck and runs ~6us/pair behind the PE, so the PE
    # work should be back-loaded to meet it at the tail
    fillers = [
        lambda: tuple(emit_v(wvh0, 0, t) for t in range(5)),
        lambda: (tuple(emit_v(wvh0, 0, t) for t in range(5, 8)),
                 emit_vt(wvh0, 0), emit_vt(wvh1, 1), emit_cvec(),
                 emit_q(2), emit_k(2)),
        lambda: (emit_q(3), emit_k(3)),
        lambda: (emit_q(4), emit_k(4)),
        lambda: (emit_q(5), emit_k(5),
                 emit_v(wvh1, 1, 0), emit_v(wvh1, 1, 1)),
        lambda: (tuple(emit_v(wvh1, 1, t) for t in range(2, 8)),
                 emit_q(6), emit_k(6)),
        lambda: (emit_q(7), emit_k(7)),
        lambda: None,
    ]

    p_attn = tc.alloc_tile_pool(name="p_attn", bufs=2)

    def emit_scores(j):
        pp = [p_attn.tile([128, KT, NTOK], FP8, tag="pp", bufs=4,
                          name=f"pp{hh}") for hh in range(2)]
        ppt = p_attn.tile([128, NTOK], FP8, tag="ppt", bufs=2, name="ppt")
        for kt in range(KT):
            ps2 = [ps_sc.tile([128, 1024], F32, tag="sc",
                              name=f"ps2{hh}") for hh in range(2)]
            for s in range(2):
                for hh in range(2):
                    base = hh * 64
                    nc.tensor.matmul(
                        ps2[hh][:, s * 512:(s + 1) * 512],
                        kT[base:base + 64, j, kt * 128:(kt + 1) * 128],
                        qT[base:base + 64, j, s * 512:(s + 1) * 512],
                        start=True, stop=True, tile_position=(base, 0),
                        skip_group_check=True)
            for hh in range(2):
                nc.scalar.activation(
                    out=pp[hh][:, kt, :], in_=ps2[hh],
                    func=AF.Exp, scale=s16_col[:, kt:kt + 1])
        # text keys: both heads packed into one [128, NTOK] psum + one exp
        ps2t = ps_sc.tile([128, 1024], F32, tag="sc", name="ps2t")
        for s in range(2):
            for hh in range(2):
                base = hh * 64
                nc.tensor.matmul(
                    ps2t[base:base + PT, s * 512:(s + 1) * 512],
                    kT[base:base + 64, j, N:N + PT],
                    qT[base:base + 64, j, s * 512:(s + 1) * 512],
                    start=True, stop=True, tile_position=(base, base),
                    skip_group_check=True)
        nc.scalar.activation(out=ppt, in_=ps2t, func=AF.Exp, scale=s3_col2)
        return pp, ppt

    def emit_av(j, pp, ppt, fast_rb=False):
        se_pr = p_attn.tile([128, NTOK], F32, tag="se_pr", name="se_pr",
                            bufs=1)
        se_r = p_attn.tile([128, NTOK], F32, tag="se_r", name="se_r",
                           bufs=1)
        for hh in range(2):
            h = 2 * j + hh
            base = hh * 64
            for s in range(2):
                ps = mm_psum()[:HD + 1, :]
                for kp in range(KT // 2):
                    nc.tensor.matmul(
                        ps, v_sb[:, 2 * kp:2 * kp + 2, h, :],
                        pp[hh][:, 2 * kp:2 * kp + 2,
                               s * 512:(s + 1) * 512],
                        start=(kp == 0), stop=False, perf_mode=DROW,
                        skip_group_check=True)
                nc.tensor.matmul(
                    ps, vt2[base:base + PT, h, :],
                    ppt[base:base + PT, s * 512:(s + 1) * 512],
                    start=False, stop=True, tile_position=(base, 0),
                    skip_group_check=True)
                nc.vector.tensor_copy(
                    out=o_lnT[base:base + 64, j, s * 512:(s + 1) * 512],
                    in_=ps[0:HD, :])
                nc.vector.tensor_copy(
                    out=se_pr[hh * 64:hh * 64 + 1, s * 512:(s + 1) * 512],
                    in_=ps[HD:HD + 1, :])
        # one call spanning partitions 0..64: the custom-DVE lowering
        # mishandles a nonzero partition offset (row 64 alone comes out
        # garbage), but a 65-partition AP starting at 0 is fine; the
        # unwritten partitions 1..63 are never read.
        nc.vector.reciprocal_approx_fast(out=se_r[0:65, :],
                                         in_=se_pr[0:65, :])
        if j == 0 and "se0" in dbg_d:
            nc.sync.dma_start(out=dbg_d["se0"][0:1, :], in_=se_pr[0:1, :])
            nc.sync.dma_start(out=dbg_d["se0"][1:2, :], in_=se_pr[64:65, :])
        if fast_rb:
            # broadcast 1/se across partitions on the PE (no DRAM round
            # trip) — used for the last pairs where the bounce latency
            # would stall the proj behind the attention tail
            se_rb = p_attn.tile([128, NTOK], BF16, tag="se_rb",
                                name="se_rb", bufs=2)
            nc.vector.tensor_copy(out=se_rb[0:65, :], in_=se_r[0:65, :])
            rbs = []
            for s in range(2):
                sl = slice(s * 512, (s + 1) * 512)
                pr = mm_psum()
                nc.tensor.matmul(pr[0:64, :], ones2[0:1, :], se_rb[0:1, sl],
                                 start=True, stop=True, tile_position=(0, 0),
                                 skip_group_check=True)
                nc.tensor.matmul(pr[64:128, :], ones2[64:65, :],
                                 se_rb[64:65, sl],
                                 start=True, stop=True,
                                 tile_position=(64, 64),
                                 skip_group_check=True)
                rbs.append(pr)
            return tuple(rbs)
        rb = p_attn.tile([128, NTOK], F32, tag="rb", name="rb", bufs=2)
        seb = dram.tile([2, NTOK], F32, tag="seb", bufs=3, name="seb")
        nc.sync.dma_start(out=seb[0:1, :], in_=se_r[0:1, :])
        nc.sync.dma_start(out=seb[1:2, :], in_=se_r[64:65, :])
        nc.sync.dma_start(
            out=rb[0:64, :], in_=seb[0:1, :].to_broadcast((64, NTOK)))
        nc.sync.dma_start(
            out=rb[64:128, :], in_=seb[1:2, :].to_broadcast((64, NTOK)))
        return rb

    def emit_normalize(j, rb):
        oj = o_lnT[:, j, :]
        if isinstance(rb, tuple):
            for s in range(2):
                sl = slice(s * 512, (s + 1) * 512)
                nc.vector.tensor_tensor(oj[:, sl], oj[:, sl], rb[s],
                                        ALU.mult)
        else:
            if j == 0 and "rb0" in dbg_d:
                nc.sync.dma_start(out=dbg_d["rb0"][:], in_=rb[:])
            nc.vector.tensor_tensor(oj, oj, rb, ALU.mult)
        nc.vector.tensor_scalar_add(out=oj, in0=oj,
                                    scalar1=c_st[:, j:j + 1])
        sq_t4 = sqp.tile([128, NTOK], BF16, tag="sq_t", name="sq_t4")
        nc.vector.tensor_tensor(sq_t4, oj, oj, ALU.mult)
        for s in range(2):
            ps_a = mm_psum()[:1, :]
            nc.tensor.matmul(
                ps_a, ones_b, oj[:, s * 512:(s + 1) * 512],
                start=True, stop=True, skip_group_check=True)
            nc.vector.tensor_tensor(
                sumO_row[:, s * 512:(s + 1) * 512],
                sumO_row[:, s * 512:(s + 1) * 512], ps_a, ALU.add)
            ps_b = mm_psum()[:1, :]
            nc.tensor.matmul(
                ps_b, ones_b, sq_t4[:, s * 512:(s + 1) * 512],
                start=True, stop=True, skip_group_check=True)
            nc.vector.tensor_tensor(
                sqO_row[:, s * 512:(s + 1) * 512],
                sqO_row[:, s * 512:(s + 1) * 512], ps_b, ALU.add)

    pend_av = None
    pend_norm = None
    for j in range(8):
        pp, ppt = emit_scores(j)
        fillers[j]()
        if pend_av is not None:
            ja, ppa, ppta = pend_av
            fast = ja >= 6
            rb = emit_av(ja, ppa, ppta, fast_rb=fast)
            if pend_norm is not None:
                emit_normalize(*pend_norm)
                pend_norm = None
            if fast:
                # rb lives in psum: normalize before the pool recycles it
                emit_normalize(ja, rb)
            else:
                pend_norm = (ja, rb)
        pend_av = (j, pp, ppt)
    ja, ppa, ppta = pend_av
    rb = emit_av(ja, ppa, ppta, fast_rb=True)
    if pend_norm is not None:
        emit_normalize(*pend_norm)
    emit_normalize(ja, rb)

    if "qT" in dbg_d:
        nc.sync.dma_start(out=dbg_d["qT"][:], in_=qT[:])
    if "kT" in dbg_d:
        nc.sync.dma_start(out=dbg_d["kT"][:], in_=kT[:])
    if "v" in dbg_d:
        nc.sync.dma_start(out=dbg_d["v"][:], in_=v_sb[:])
    if "v_text" in dbg_d:
        nc.sync.dma_start(out=dbg_d["v_text"][:], in_=vt_sb[:])
    if "o_lnT" in dbg_d:
        nc.sync.dma_start(out=dbg_d["o_lnT"][:], in_=o_lnT[:])

    # free attention-only SBUF (LIFO), then bring in the late pools
    p_attn.release()
    p_wvh.release()
    p_wblk.release()
    p_qkv.release()
    colp.release()
    ebc.release()
    p_xtb.release()

    # =====================================================================
    # subln rows (deferred into proj): ssub_b, nsub_b [128, NTOK]
    # =====================================================================
    lbc = tc.alloc_tile_pool(name="lbc", bufs=1)
    ssub_b = lbc.tile([128, NTOK], F32, name="ssub_b")
    nsub_b = lbc.tile([128, NTOK], F32, name="nsub_b")
    m2_b = lbc.tile([128, NTOK], F32, name="m2_b")
    s2_b = lbc.tile([128, NTOK], F32, name="s2_b")
    sffn_b = lbc.tile([128, NTOK], F32, name="sffn_b")
    nffn_b = lbc.tile([128, NTOK], F32, name="nffn_b")

    sub_bin = dram.tile([2, NTOK], F32, name="sub_bin")
    nc.sync.dma_start(out=sub_bin[0:1, :], in_=sumO_row)
    nc.sync.dma_start(out=sub_bin[1:2, :], in_=sqO_row)
    bout_s = ln_math(sub_bin, NTOK, D, "subln")
    nc.sync.dma_start(out=ssub_b,
                      in_=bout_s[2:3, :].to_broadcast((128, NTOK)))
    nc.sync.dma_start(out=nsub_b,
                      in_=bout_s[4:5, :].to_broadcast((128, NTOK)))

    # =====================================================================
    # proj + residual -> xnbT (bf16); LN2 split by token halves.
    # =====================================================================
    p_xnbT = tc.alloc_tile_pool(name="p_xnbT", bufs=1)
    xnbT = p_xnbT.tile([128, DT, NTOK], BF16, name="xnbT")
    p_x2T = tc.alloc_tile_pool(name="p_x2T", bufs=1)
    x2T = p_x2T.tile([128, DT, NTOK], BF16, name="x2T")
    p_wp = tc.alloc_tile_pool(name="p_wp", bufs=1)
    wp_sb = p_wp.tile([128, KT, D], BF16, name="wp_sb")
    nc.sync.dma_start(out=wp_sb, in_=wp_d.rearrange("k p d -> p k d"))
    skip_bp = "bp" in triv

    def emit_proj_half(s):
        sl = slice(s * HLF, (s + 1) * HLF)
        for m in range(DT):
            ps = mm_psum()
            for o in range(KT):
                nc.tensor.matmul(
                    ps, wp_sb[:, o, m * 128:(m + 1) * 128], o_lnT[:, o, sl],
                    start=(o == 0), stop=(o == KT - 1))
            t1 = tmps.tile([128, 512], F32, tag="wrk", name="pj_t1")
            nc.vector.tensor_tensor(t1, ps, ssub_b[:, sl], ALU.mult)
            nc.vector.scalar_tensor_tensor(
                out=t1, in0=nsub_b[:, sl], scalar=st["wpsum"][:, m:m + 1],
                in1=t1, op0=ALU.mult, op1=ALU.add)
            if not skip_bp:
                nc.vector.tensor_scalar_add(
                    out=t1, in0=t1, scalar1=st["bp"][:, m:m + 1])
            nc.vector.tensor_tensor(xnbT[:, m, sl], t1, xbT[:, m, sl],
                                    ALU.add)

    def emit_ln2_half(s):
        sl = slice(s * HLF, (s + 1) * HLF)
        t = ps_sc.tile([128, 1024], F32, tag="sc", name="ln2ps")
        ps_a, ps_b = t[:1, 0:512], t[:1, 512:1024]
        for o in range(DT):
            sq_t = sqp.tile([128, NTOK], BF16, tag="sq_t",
                            name="sq_t2")[:, :HLF]
            nc.vector.tensor_tensor(sq_t, xnbT[:, o, sl], xnbT[:, o, sl],
                                    ALU.mult)
            nc.tensor.matmul(ps_a, ones_b, xnbT[:, o, sl],
                             start=(o == 0), stop=(o == DT - 1),
                             skip_group_check=True)
            nc.tensor.matmul(ps_b, ones_b, sq_t,
                             start=(o == 0), stop=(o == DT - 1),
                             skip_group_check=True)
        bout2 = ln_chain(ps_a, ps_b, HLF, D, f"ln2_{s}")
        nc.sync.dma_start(out=m2_b[:, sl],
                          in_=bout2[0:1, :HLF].to_broadcast((128, HLF)))
        nc.sync.dma_start(out=s2_b[:, sl],
                          in_=bout2[2:3, :HLF].to_broadcast((128, HLF)))
        for o in range(DT):
            tA = tmps.tile([128, 512], F32, tag="wrk2", name="ln2_tA",
                           bufs=2)
            nc.vector.tensor_tensor(tA, xnbT[:, o, sl], m2_b[:, sl],
                                    ALU.subtract)
            nc.vector.tensor_tensor(x2T[:, o, sl], tA, s2_b[:, sl],
                                    ALU.mult)

    emit_proj_half(0)
    emit_ln2_half(0)
    emit_proj_half(1)
    emit_ln2_half(1)
    p_wp.release()
    if "xnbT" in dbg_d:
        nc.sync.dma_start(out=dbg_d["xnbT"][:], in_=xnbT[:])
    if "x2T" in dbg_d:
        nc.sync.dma_start(out=dbg_d["x2T"][:], in_=x2T[:])

    # =====================================================================
    # fc1 + gelu -> hT; ffn stats (hierarchical bf16 accumulation on DVE);
    # ffn rows split by halves, deferred into fc2.
    # =====================================================================
    p_hT = tc.alloc_tile_pool(name="p_hT", bufs=1, side="right")
    hT = p_hT.tile([128, HT, NTOK], BF16, name="hT")

    skip_b1 = "b1" in triv

    p_facc = tc.alloc_tile_pool(name="p_facc", bufs=1)
    facc_s = p_facc.tile([128, NTOK], F32, name="facc_s")
    facc_q = p_facc.tile([128, NTOK], F32, name="facc_q")
    f16_s = p_facc.tile([128, NTOK], BF16, name="f16_s")
    f16_q = p_facc.tile([128, NTOK], BF16, name="f16_q")
    p_w1blk = tc.alloc_tile_pool(name="p_w1blk", bufs=3)

    nc.vector.memset(facc_s, 0.0)
    nc.vector.memset(facc_q, 0.0)
    nc.vector.memset(f16_s, 0.0)
    nc.vector.memset(f16_q, 0.0)
    # the first eight hm blocks run s=0 only, so the LN2(s1) row chain can
    # finish behind the s=0 matmuls without idling the PE
    fc1_order = [(hm, 0) for hm in range(8)] \
        + [(hm, 1) for hm in range(8)] \
        + [(hm, s) for hm in range(8, HT) for s in range(2)]
    last_hm = -1
    w1blk = None
    cnt = {0: 0, 1: 0}
    for hm, s in fc1_order:
        if hm != last_hm:
            w1blk = p_w1blk.tile([128, KT, 128], BF16, tag="w1blk",
                                 name="w1blk")
            nc.sync.dma_start(out=w1blk, in_=w1_d[hm])
            last_hm = hm
        sl = slice(s * HLF, (s + 1) * HLF)
        ps = mm_psum()
        for o in range(KT):
            nc.tensor.matmul(
                ps, w1blk[:, o, :], x2T[:, o, sl],
                start=(o == 0), stop=(o == KT - 1))
        hslice = hT[:, hm, sl]
        if skip_b1:
            nc.scalar.activation(out=hslice, in_=ps, func=AF.Gelu)
        else:
            nc.scalar.activation(
                out=hslice, in_=ps, func=AF.Gelu,
                bias=st["b1"][:, hm:hm + 1], scale=1.0)
        sq_t = sqp.tile([128, NTOK], BF16, tag="sq_t",
                        name="sq_tf")[:, :HLF]
        nc.vector.tensor_tensor(sq_t, hslice, hslice, ALU.mult)
        nc.vector.tensor_tensor(f16_s[:, sl], f16_s[:, sl], hslice,
                                ALU.add)
        nc.vector.tensor_tensor(f16_q[:, sl], f16_q[:, sl], sq_t,
                                ALU.add)
        cnt[s] += 1
        if cnt[s] % 8 == 0:
            nc.vector.tensor_tensor(facc_s[:, sl], facc_s[:, sl],
                                    f16_s[:, sl], ALU.add)
            nc.vector.tensor_tensor(facc_q[:, sl], facc_q[:, sl],
                                    f16_q[:, sl], ALU.add)
            nc.vector.memset(f16_s[:, sl], 0.0)
            nc.vector.memset(f16_q[:, sl], 0.0)

    def emit_ffn_rows(s):
        sl = slice(s * HLF, (s + 1) * HLF)
        cst_s = sqp.tile([128, NTOK], BF16, tag="cst", name="cst_s")[:, :HLF]
        cst_q = sqp.tile([128, NTOK], BF16, tag="sq_t", name="cst_q")[:, :HLF]
        nc.scalar.copy(out=cst_s, in_=facc_s[:, sl])
        nc.scalar.copy(out=cst_q, in_=facc_q[:, sl])
        t = ps_sc.tile([128, 1024], F32, tag="sc", name="ffnps")
        ps_a, ps_b = t[:1, 0:512], t[:1, 512:1024]
        nc.tensor.matmul(ps_a, ones_b, cst_s, start=True, stop=True,
                         skip_group_check=True)
        nc.tensor.matmul(ps_b, ones_b, cst_q, start=True, stop=True,
                         skip_group_check=True)
        boutf = ln_chain(ps_a, ps_b, HLF, HID, f"ffn_{s}")
        nc.sync.dma_start(out=sffn_b[:, sl],
                          in_=boutf[2:3, :HLF].to_broadcast((128, HLF)))
        nc.sync.dma_start(out=nffn_b[:, sl],
                          in_=boutf[4:5, :HLF].to_broadcast((128, HLF)))

    emit_ffn_rows(0)
    emit_ffn_rows(1)
    p_w1blk.release()
    p_facc.release()
    p_x2T.release()
    if "hT" in dbg_d:
        nc.sync.dma_start(out=dbg_d["hT"][:], in_=hT[:])

    # =====================================================================
    # fc2 (deferred ffn_ln) + residual + transpose + store
    # =====================================================================
    skip_b2 = "b2" in triv
    with tc.tile_pool(name="p_w2blk", bufs=3) as p_w2blk, \
         tc.tile_pool(name="p_out", bufs=2) as p_out, \
         tc.tile_pool(name="p_stg2", bufs=4) as p_stg2:
        def store_out(outm, m, s):
            for t in range(4):
                pst = mm_psum(128)
                nc.tensor.transpose(
                    pst, outm[:, t * 128:(t + 1) * 128], ident)
                stg = p_stg2.tile([128, 128], F32, tag="stg", name="stg2")
                nc.scalar.copy(out=stg, in_=pst)
                nc.sync.dma_start(
                    out=y_d[(s * 4 + t) * 128:(s * 4 + t + 1) * 128,
                            m * 128:(m + 1) * 128],
                    in_=stg)

        pend_out = None  # transpose/store lags one group so the PE never
        for m in range(DT):  # waits on the DVE epilogue of the current one
            w2blk = p_w2blk.tile([128, HT, 128], BF16, tag="w2blk",
                                 name="w2blk")
            nc.sync.dma_start(out=w2blk, in_=w2_d[m])
            for s in range(2):
                sl = slice(s * HLF, (s + 1) * HLF)
                outm = p_out.tile([128, 512], F32, tag="outm", name="outm")
                ps = mm_psum()
                for o in range(HT):
                    nc.tensor.matmul(
                        ps, w2blk[:, o, :], hT[:, o, sl],
                        start=(o == 0), stop=(o == HT - 1))
                t1 = tmps.tile([128, 512], F32, tag="wrk", name="f2_t1")
                nc.vector.tensor_tensor(t1, ps, sffn_b[:, sl], ALU.mult)
                nc.vector.scalar_tensor_tensor(
                    out=t1, in0=nffn_b[:, sl], scalar=st["w2sum"][:, m:m + 1],
                    in1=t1, op0=ALU.mult, op1=ALU.add)
                if not skip_b2:
                    nc.vector.tensor_scalar_add(
                        out=t1, in0=t1, scalar1=st["b2"][:, m:m + 1])
                nc.vector.tensor_tensor(outm, t1, xnbT[:, m, sl], ALU.add)
                if pend_out is not None:
                    store_out(*pend_out)
                pend_out = (outm, m, s)
        store_out(*pend_out)
    p_hT.release()
    p_OlnT.release()
    p_xnbT.release()
    lbc.release()
    p_xbT.release()
    ctx.close()


# --------------------------------------------------------------------------
# host glue
# --------------------------------------------------------------------------

_PROGRAM_CACHE = {}


def get_program(debug=(), triv=frozenset()):
    key = (tuple(sorted(debug)), tuple(sorted(triv)))
    if key not in _PROGRAM_CACHE:
        _PROGRAM_CACHE[key] = build_program(debug=key[0], triv=key[1])
    return _PROGRAM_CACHE[key]


def _host_fold(inputs):
    """Fold LN gains/biases into weights; compute effective biases and the
    column sums used by the deferred-LN epilogues."""
    f32 = np.float32

    def host(name):
        return np.asarray(inputs[name], dtype=f32)

    g1 = host("n1_g")
    b1n = host("n1_b")
    g3 = host("n3_g")
    b3n = host("n3_b")
    if not (np.array_equal(g1, g3) and np.array_equal(b1n, b3n)):
        assert np.all(g1 == 1.0) and np.all(b1n == 0.0) \
            and np.all(g3 == 1.0) and np.all(b3n == 0.0), \
            "distinct non-trivial n1/n3 gains unsupported"
    wq_eff = g1[:, None] * host("Wq")
    wk_eff = g1[:, None] * host("Wk")
    wv_eff = g1[:, None] * host("Wv")
    bq_eff = host("bq") + b1n @ host("Wq")
    kb_eff = b1n @ host("Wk")
    bv_eff = host("bv") + b1n @ host("Wv")
    wp_eff = host("ln_g")[:, None] * host("Wp")
    bp_eff = host("bp") + host("ln_b") @ host("Wp")
    g2 = host("n2_g")
    b2n = host("n2_b")
    w1_eff = g2[:, None] * host("W1")
    b1_eff = host("b1") + b2n @ host("W1")
    w2_eff = host("ffn_g")[:, None] * host("W2")
    b2_eff = host("b2") + host("ffn_b") @ host("W2")
    return dict(wq_eff=wq_eff, wk_eff=wk_eff, wv_eff=wv_eff,
                bq_eff=bq_eff, kb_eff=kb_eff, bv_eff=bv_eff,
                wp_eff=wp_eff, bp_eff=bp_eff,
                w1_eff=w1_eff, b1_eff=b1_eff,
                w2_eff=w2_eff, b2_eff=b2_eff)


def compute_triv(inputs):
    e = _host_fold(inputs)
    triv = set()
    for k, v in [("bq", e["bq_eff"]), ("kb", e["kb_eff"]),
                 ("bv", e["bv_eff"]), ("bp", e["bp_eff"]),
                 ("b1", e["b1_eff"]), ("b2", e["b2_eff"])]:
        if np.all(v == 0.0):
            triv.add(k)
    return frozenset(triv)


def make_in_maps(inputs):
    bf = ml_dtypes.bfloat16
    f32 = np.float32
    e = _host_fold(inputs)

    fp8 = ml_dtypes.float8_e4m3
    wq_f8 = (e["wq_eff"] * 16.0).astype(fp8)
    wk_f8 = (e["wk_eff"] * 16.0).astype(fp8)
    wv_bf = e["wv_eff"].astype(bf)
    wp_bf = e["wp_eff"].astype(bf)
    w1_bf = e["w1_eff"].astype(bf)
    w2_bf = e["w2_eff"].astype(bf)
    shared = dict(
        wq=np.ascontiguousarray(
            wq_f8.reshape(KT, 128, DT, 128).transpose(2, 1, 0, 3)),
        wk=np.ascontiguousarray(
            wk_f8.reshape(KT, 128, DT, 128).transpose(2, 1, 0, 3)),
        wv=np.ascontiguousarray(
            wv_bf.reshape(KT, 128, 2, 512).transpose(2, 1, 0, 3)),
        wp=wp_bf.reshape(KT, 128, D),
        w1=np.ascontiguousarray(
            w1_bf.reshape(KT, 128, HT, 128).transpose(2, 1, 0, 3)),
        w2=np.ascontiguousarray(
            w2_bf.reshape(HT, 128, DT, 128).transpose(2, 1, 0, 3)),
        wqsum=wq_f8.astype(f32).sum(0),
        wksum=wk_f8.astype(f32).sum(0),
        wvsum=wv_bf.astype(f32).sum(0),
        wpsum=wp_bf.astype(f32).sum(0),
        w2sum=w2_bf.astype(f32).sum(0),
        bq8=(e["bq_eff"] * 0.125).astype(f32),
        kb=(e["kb_eff"] * 16.0).astype(f32),
        bv_eff=e["bv_eff"].astype(f32),
        bp=e["bp_eff"].astype(f32),
        b1=e["b1_eff"].astype(f32),
        b2=e["b2_eff"].astype(f32),
        ident=np.eye(128, dtype=f32),
        gate=np.asarray(inputs["gate"], f32).reshape(H),
    )
    x = np.asarray(inputs["x"], f32)
    xt = np.asarray(inputs["x_text"], f32)
    in_maps = []
    for b in range(B):
        m = dict(shared)
        m["x"] = np.ascontiguousarray(x[b])
        m["x_text"] = np.ascontiguousarray(xt[b])
        in_maps.append(m)
    return in_maps


def kernel(**inputs) -> np.ndarray:
    from concourse.bass_utils import run_bass_kernel_spmd

    nc = get_program(triv=compute_triv(inputs))
    in_maps = make_in_maps(inputs)
    res = run_bass_kernel_spmd(nc, in_maps, list(range(B)))
    out = np.stack([np.asarray(res.results[b]["y"]) for b in range(B)], axis=0)
    return out.astype(np.float32)


# revision 111
# speedup vs baseline: 1.0559x; 1.0559x over previous
"""Trainium2 Bass kernel for nn_Block_45724221833665 (dense transformer block).

Strategy: pure data-parallel over batch — 8 batch elements, 8 NeuronCores, one
batch element per core, no collectives.  Feature-major compute throughout.

v2 rewrite, engineered around the PE p-state ramp (the tensor engine runs at
1.2 GHz until it has been continuously busy for 3 us, 2.4 GHz after): the
emission order is arranged so the PE never stalls.

  * LN1/LN3 are DEFERRED into the Q/K/V projections: the matmuls run on the
    raw transposed input (bf16), and the normalization becomes a fused psum
    epilogue  out = s[t] * (P + (-m[t]) * wsum[d])  on DVE
    (scalar_tensor_tensor + tensor_tensor).  The key-side rstd is folded into
    the EXP activation's per-partition scale AP, so kT needs no epilogue
    multiply at all.  The PE starts projecting immediately after the
    transposes; the LN row computation overlaps the first matmuls.
  * The softmax denominators use DVE reciprocal_approx_fast instead of the
    ACT-table reciprocal: ACT keeps the exp table loaded for the whole
    attention phase (no table thrash) and stays exp-only there (it is the
    attention-phase bottleneck at ~18us/pair).
  * Q/K projections for head pairs 2..7 and the second half of V are
    interleaved INTO the attention pair loop, filling the PE while ACT chews
    the exp stream.
  * LN2 and the ffn_ln run split by token halves: stats/rows/apply for tokens
    0-511 overlap the second half's matmuls (proj s1, fc1 s1), so the row
    latency never idles the PE.  subln and ffn_ln stay deferred into the
    following matmul (rank-1 mean correction + rstd scale in the epilogue).
  * The residual uses xbT (bf16 x^T) directly — no f32 x bounce to DRAM; the
    post-attention state xn is kept in bf16 only.
  * W1 is resident in SBUF (DMA'd during proj, after attention frees SBUF);
    W1/W2 are each streamed from HBM exactly once.
"""

import numpy as np
import ml_dtypes

import concourse.bass as bass
import concourse.mybir as mybir
import concourse.tile as tile
from concourse import bacc

B, N, PT, D, H, HD, HID = 8, 1024, 64, 1024, 16, 64, 4096
KT = D // 128          # 8 contraction tiles over D
HT = HID // 128        # 32 tiles over HID
DT = D // 128          # 8 output tiles over D
NTOK = N               # 1024 main tokens per core
NKEY = N + PT          # 1088 keys (main tokens + text prefix)
HLF = NTOK // 2        # 512 token half
EPS = 1e-5

F32 = mybir.dt.float32
BF16 = mybir.dt.bfloat16
FP8 = mybir.dt.float8e4
AF = mybir.ActivationFunctionType
ALU = mybir.AluOpType
DROW = mybir.MatmulPerfMode.DoubleRow


def build_program(debug=(), triv=frozenset()):
    nc = bacc.Bacc("TRN2")
    dbg = set(debug)
    triv = set(triv)

    # ---- I/O ------------------------------------------------------------
    x_d = nc.declare_dram_parameter("x", [NTOK, D], F32, isOutput=False)
    xt_d = nc.declare_dram_parameter("x_text", [PT, D], F32, isOutput=False)
    # Q/K weights blocked by output tile m; V blocked by output half sn.
    # Q/K run in fp8 DoubleRow (host-prescaled by 16 out of e4m3's denormal
    # range; the 1/16 is folded into the s8 row / the exp scale row) — their
    # elementwise errors average out in the softmax.
    wq_d = nc.declare_dram_parameter("wq", [DT, 128, KT, 128], FP8,
                                     isOutput=False)
    wk_d = nc.declare_dram_parameter("wk", [DT, 128, KT, 128], FP8,
                                     isOutput=False)
    wv_d = nc.declare_dram_parameter("wv", [2, 128, KT, 512], BF16,
                                     isOutput=False)
    wp_d = nc.declare_dram_parameter("wp", [KT, 128, D], BF16, isOutput=False)
    w1_d = nc.declare_dram_parameter("w1", [HT, 128, KT, 128], BF16,
                                     isOutput=False)
    w2_d = nc.declare_dram_parameter("w2", [DT, 128, HT, 128], BF16,
                                     isOutput=False)
    pvec = {}
    for nm, sz in [
        ("bq8", D), ("bp", D), ("b2", D), ("kb", D), ("bv_eff", D),
        ("b1", HID),
        ("wqsum", D), ("wksum", D), ("wvsum", D), ("wpsum", D), ("w2sum", D),
    ]:
        pvec[nm] = nc.declare_dram_parameter(nm, [sz], F32, isOutput=False)
    ident_d = nc.declare_dram_parameter("ident", [128, 128], F32,
                                        isOutput=False)
    gate_d = nc.declare_dram_parameter("gate", [H], F32, isOutput=False)
    y_d = nc.declare_dram_parameter("y", [NTOK, D], F32, isOutput=True)

    dbg_d = {}

    def dbg_out(name, shape, dtype):
        if name in dbg:
            dbg_d[name] = nc.declare_dram_parameter(
                "dbg_" + name, list(shape), dtype, isOutput=True
            )

    dbg_out("xbT", [128, DT, NTOK], BF16)
    dbg_out("qT", [128, DT, NTOK], BF16)
    dbg_out("kT", [128, DT, NKEY], BF16)
    dbg_out("v", [128, KT, H, HD + 1], FP8)
    dbg_out("v_text", [PT, H, HD + 1], FP8)
    dbg_out("c_vec", [D], F32)
    dbg_out("o_lnT", [128, DT, NTOK], BF16)
    dbg_out("pp00", [128, KT, NTOK], BF16)
    dbg_out("se0", [2, NTOK], F32)
    dbg_out("rb0", [128, NTOK], F32)
    dbg_out("opr0", [128, NTOK], F32)
    dbg_out("xnbT", [128, DT, NTOK], BF16)
    dbg_out("x2T", [128, DT, NTOK], BF16)
    dbg_out("hT", [128, HT, NTOK], BF16)

    with tile.TileContext(nc) as tc:
        _build(nc, tc, x_d, xt_d, wq_d, wk_d, wv_d, wp_d,
               w1_d, w2_d, pvec, gate_d, y_d, dbg_d, triv, ident_d)
    nc.compile()
    return nc


def _build(nc, tc, x_d, xt_d, wq_d, wk_d, wv_d, wp_d, w1_d, w2_d,
           pvec, gate_d, y_d, dbg_d, triv, ident_d):
    import contextlib
    ctx = contextlib.ExitStack()
    consts = ctx.enter_context(tc.tile_pool(name="consts", bufs=1))
    rows = ctx.enter_context(tc.tile_pool(name="rows", bufs=1))
    tmps = ctx.enter_context(tc.tile_pool(name="tmps", bufs=3))
    sqp = ctx.enter_context(tc.tile_pool(name="sqp", bufs=2))
    rwp = ctx.enter_context(tc.tile_pool(name="rwp", bufs=2))
    dram = ctx.enter_context(tc.tile_pool(name="dram", bufs=1, space="DRAM"))
    ps_mm = ctx.enter_context(tc.tile_pool(name="ps_mm", bufs=4, space="PSUM"))
    ps_sc = ctx.enter_context(tc.tile_pool(name="ps_sc", bufs=2, space="PSUM"))

    def mm_psum(pfree=512, parts=128):
        t = ps_mm.tile([128, 512], F32, tag="mm", name="mmps")
        return t[:parts, :pfree]

    # ---- constants / parameter DMAs ------------------------------------
    ident = consts.tile([128, 128], F32, name="ident")
    nc.sync.dma_start(out=ident, in_=ident_d[:, :])
    ones_b = consts.tile([128, 1], BF16, name="ones_b")
    nc.vector.memset(ones_b, 1.0)
    ones2 = consts.tile([128, 64], BF16, name="ones2")
    nc.vector.memset(ones2, 1.0)
    eps_c = consts.tile([64, 1], F32, name="eps_c")
    nc.vector.memset(eps_c, EPS)

    st = {}
    for nm in ["bq8", "bp", "b2", "kb", "wqsum", "wksum", "wpsum", "w2sum"]:
        t = consts.tile([128, DT], F32, name="st_" + nm)
        nc.sync.dma_start(out=t, in_=pvec[nm].rearrange("(o p) -> p o", p=128))
        st[nm] = t
    t = consts.tile([128, HT], F32, name="st_b1")
    nc.sync.dma_start(out=t, in_=pvec["b1"].rearrange("(o p) -> p o", p=128))
    st["b1"] = t

    # xbT sits at the bottom of the left stack (lives until the proj
    # residual); everything allocated above it is released in LIFO order.
    p_xbT = tc.alloc_tile_pool(name="p_xbT", bufs=1)
    xbT = p_xbT.tile([128, DT, NTOK], BF16, name="xbT")
    p_xtb = tc.alloc_tile_pool(name="p_xtb", bufs=1)
    xtb = p_xtb.tile([128, DT, PT], BF16, name="xtb")
    # fp8 copies of x^T / x_text^T for the fp8 Q/K projections
    xb8 = p_xtb.tile([128, DT, NTOK], FP8, name="xb8")
    xtb8 = p_xtb.tile([128, DT, PT], FP8, name="xtb8")

    # early broadcast pool (released after the QKV projections)
    ebc = tc.alloc_tile_pool(name="ebc", bufs=1)
    wvsum_b = ebc.tile([128, D], F32, name="wvsum_b")
    nc.sync.dma_start(
        out=wvsum_b,
        in_=pvec["wvsum"].rearrange("(a d) -> a d", a=1).to_broadcast((128, D)))
    negm_b = ebc.tile([128, NTOK], F32, name="negm_b")
    s8_b = ebc.tile([128, NTOK], F32, name="s8_b")
    negm3_b = ebc.tile([128, PT], F32, name="negm3_b")
    # small striped columns (alive through attention)
    colp = tc.alloc_tile_pool(name="colp", bufs=1)
    s_col = colp.tile([128, KT], F32, name="s_col")
    s16_col = colp.tile([128, KT], F32, name="s16_col")
    nm_col = colp.tile([128, KT], F32, name="nm_col")
    s3_col = colp.tile([PT, 1], F32, name="s3_col")
    nm3_col = colp.tile([PT, 1], F32, name="nm3_col")
    s3_col2 = colp.tile([128, 1], F32, name="s3_col2")

    # ---- LN row machinery -----------------------------------------------
    # Row math runs in [64, w//64] layout (engine APs must start at
    # partition 0/64, and a [1, w] tile reserves w*4 bytes on EVERY
    # partition — the 2D layout costs next to nothing).  PSUM stat rows are
    # staged through a [1, 2*HLF] tile, bounced to DRAM (DMA reshapes
    # freely), mathed, and the result rows land in a [5, w] DRAM tile:
    # row 0=mean, 1=-mean, 2=rstd, 3=rstd/8, 4=-mean*rstd.
    stage = rows.tile([1, 2 * HLF], F32, name="stage")

    def ln_chain(ps_a, ps_b, w, n_elems, name, s_scale=None):
        """ps_a/ps_b: [1, w] APs (psum or sbuf rows) with sum / sum-of-sq.
        Returns a [5, w] DRAM tile (mean, negm, s, s8, ns rows)."""
        nc.vector.tensor_copy(out=stage[:, 0:w], in_=ps_a)
        nc.vector.tensor_copy(out=stage[:, w:2 * w], in_=ps_b)
        bin_ = dram.tile([2, NTOK], F32, tag="lnbin", bufs=2, name=name + "i")
        nc.sync.dma_start(out=bin_[0:1, :w], in_=stage[:, 0:w])
        nc.sync.dma_start(out=bin_[1:2, :w], in_=stage[:, w:2 * w])
        return ln_math(bin_, w, n_elems, name, s_scale)

    def ln_math(bin_, w, n_elems, name, s_scale=None):
        wf = w // 64
        r = rwp.tile([64, 9, 16], F32, tag="rw", name=name + "r")
        r_sum, r_sq, r_t, r_u = (r[:, i, :wf] for i in range(4))
        r_negm, r_s, r_s8, r_ns = (r[:, i, :wf] for i in range(4, 8))
        r_s16 = r[:, 8, :wf]
        nc.sync.dma_start(
            out=r_sum, in_=bin_[0:1, :w].rearrange("a (p f) -> (a p) f", p=64))
        nc.sync.dma_start(
            out=r_sq, in_=bin_[1:2, :w].rearrange("a (p f) -> (a p) f", p=64))
        inv = 1.0 / float(n_elems)
        nc.vector.tensor_scalar_mul(out=r_sum, in0=r_sum, scalar1=inv)
        nc.vector.tensor_scalar_mul(out=r_sq, in0=r_sq, scalar1=inv)
        nc.vector.tensor_tensor(r_t, r_sum, r_sum, ALU.mult)
        nc.vector.tensor_tensor(r_sq, r_sq, r_t, ALU.subtract)  # var
        nc.scalar.activation(out=r_s, in_=r_sq, func=AF.Abs_reciprocal_sqrt,
                             bias=eps_c, scale=1.0)
        # rsqrt Newton: s <- s*(1.5 - 0.5*(var+eps)*s^2)
        nc.vector.tensor_scalar_add(out=r_t, in0=r_sq, scalar1=EPS)
        nc.vector.tensor_tensor(r_u, r_s, r_s, ALU.mult)
        nc.vector.tensor_tensor(r_u, r_u, r_t, ALU.mult)
        nc.vector.tensor_scalar(out=r_u, in0=r_u, scalar1=-0.5,
                                scalar2=1.5, op0=ALU.mult, op1=ALU.add)
        nc.vector.tensor_tensor(r_s, r_s, r_u, ALU.mult)
        nc.vector.tensor_scalar_mul(out=r_negm, in0=r_sum, scalar1=-1.0)
        # s8 row absorbs the 1/16 un-scaling of the fp8 Wq (0.125/16)
        nc.vector.tensor_scalar_mul(out=r_s8, in0=r_s, scalar1=0.0078125)
        nc.vector.tensor_tensor(r_ns, r_negm, r_s, ALU.mult)
        # s16 row: rstd/16 — the exp scale un-scaling the fp8 Wk
        nc.vector.tensor_scalar_mul(out=r_s16, in0=r_s, scalar1=0.0625)
        bout = dram.tile([6, NTOK], F32, tag="lnbout", bufs=2,
                         name=name + "o")
        for i, src in enumerate([r_sum, r_negm, r_s, r_s8, r_ns, r_s16]):
            nc.sync.dma_start(
                out=bout[i:i + 1, :w].rearrange("a (p f) -> (a p) f", p=64),
                in_=src)
        return bout

    # =====================================================================
    # PH1: load x / x_text, transpose -> xbT (bf16, raw).  LN1 stats/rows
    # per token half; LN3 for text.
    # =====================================================================
    def emit_ln1_half(h, ps_a, ps_b):
        sl = slice(h * HLF, (h + 1) * HLF)
        bout = ln_chain(ps_a, ps_b, HLF, D, f"ln1_{h}")
        nc.sync.dma_start(out=negm_b[:, sl],
                          in_=bout[1:2, :HLF].to_broadcast((128, HLF)))
        nc.sync.dma_start(out=s8_b[:, sl],
                          in_=bout[3:4, :HLF].to_broadcast((128, HLF)))
        nc.sync.dma_start(
            out=s_col[:, h * 4:(h + 1) * 4],
            in_=bout[2:3, :HLF].rearrange("a (o p) -> (a p) o", p=128))
        nc.sync.dma_start(
            out=s16_col[:, h * 4:(h + 1) * 4],
            in_=bout[5:6, :HLF].rearrange("a (o p) -> (a p) o", p=128))
        nc.sync.dma_start(
            out=nm_col[:, h * 4:(h + 1) * 4],
            in_=bout[1:2, :HLF].rearrange("a (o p) -> (a p) o", p=128))

    with tc.tile_pool(name="p_x", bufs=1) as p_x:
        x_sb = p_x.tile([128, DT, D], F32, name="x_sb")
        for hf in range(4):
            nc.sync.dma_start(
                out=x_sb[:, hf * 2:(hf + 1) * 2, :],
                in_=x_d[hf * 256:(hf + 1) * 256, :].rearrange(
                    "(t p) d -> p t d", p=128))
        xt_sb = p_x.tile([PT, D], F32, name="xt_sb")
        nc.sync.dma_start(out=xt_sb, in_=xt_d[:, :])

        def stat_ps():
            t = ps_sc.tile([128, 1024], F32, tag="sc", name="statps")
            return t[:1, 0:512], t[:1, 512:1024]

        for h in range(2):  # token halves
            sl = slice(h * HLF, (h + 1) * HLF)
            for t in range(h * 4, (h + 1) * 4):
                for o in range(DT):
                    pst = mm_psum(128)
                    nc.tensor.transpose(
                        pst, x_sb[:, t, o * 128:(o + 1) * 128], ident)
                    nc.vector.tensor_copy(
                        out=xbT[:, o, t * 128:(t + 1) * 128], in_=pst)
                    nc.scalar.copy(
                        out=xb8[:, o, t * 128:(t + 1) * 128], in_=pst)
            ps_a, ps_b = stat_ps()
            for o in range(DT):
                sq_t = sqp.tile([128, NTOK], BF16, tag="sq_t",
                                name="sq_t")[:, :HLF]
                nc.vector.tensor_tensor(sq_t, xbT[:, o, sl], xbT[:, o, sl],
                                        ALU.mult)
                nc.tensor.matmul(ps_a, ones_b, xbT[:, o, sl],
                                 start=(o == 0), stop=(o == DT - 1),
                                 skip_group_check=True)
                nc.tensor.matmul(ps_b, ones_b, sq_t,
                                 start=(o == 0), stop=(o == DT - 1),
                                 skip_group_check=True)
            emit_ln1_half(h, ps_a, ps_b)

        # ---- text: transpose + LN3 rows --------------------------------
        for o in range(DT):
            pst = mm_psum(PT)
            nc.tensor.transpose(
                pst, xt_sb[:, o * 128:(o + 1) * 128], ident[:PT, :PT])
            nc.vector.tensor_copy(out=xtb[:, o, :], in_=pst)
            nc.scalar.copy(out=xtb8[:, o, :], in_=pst)
        ps_a, ps_b = stat_ps()
        ps_a = ps_a[:, :PT]
        ps_b = ps_b[:, :PT]
        for o in range(DT):
            sq_t = sqp.tile([128, NTOK], BF16, tag="sq_t",
                            name="sq_t3")[:, :PT]
            nc.vector.tensor_tensor(sq_t, xtb[:, o, :], xtb[:, o, :],
                                    ALU.mult)
            nc.tensor.matmul(ps_a, ones_b, xtb[:, o, :],
                             start=(o == 0), stop=(o == DT - 1),
                             skip_group_check=True)
            nc.tensor.matmul(ps_b, ones_b, sq_t,
                             start=(o == 0), stop=(o == DT - 1),
                             skip_group_check=True)
        bout3 = ln_chain(ps_a, ps_b, PT, D, "ln3")
        nc.sync.dma_start(out=negm3_b,
                          in_=bout3[1:2, :PT].to_broadcast((128, PT)))
        nc.sync.dma_start(
            out=s3_col,
            in_=bout3[2:3, :PT].rearrange("a (o p) -> (a p) o", p=PT))
        nc.sync.dma_start(
            out=nm3_col,
            in_=bout3[1:2, :PT].rearrange("a (o p) -> (a p) o", p=PT))
        nc.sync.dma_start(
            out=s3_col2[0:PT],
            in_=bout3[5:6, :PT].rearrange("a (o p) -> (a p) o", p=PT))
        nc.sync.dma_start(
            out=s3_col2[PT:128],
            in_=bout3[5:6, :PT].rearrange("a (o p) -> (a p) o", p=PT))

    if "xbT" in dbg_d:
        nc.sync.dma_start(out=dbg_d["xbT"][:], in_=xbT[:])

    # =====================================================================
    # Projections (deferred-LN epilogues) + attention, interleaved.
    # =====================================================================
    p_qkv = tc.alloc_tile_pool(name="p_qkv", bufs=1)
    qT = p_qkv.tile([128, DT, NTOK], BF16, name="qT")
    kT = p_qkv.tile([128, DT, NKEY], BF16, name="kT")
    v_sb = p_qkv.tile([128, KT, H, HD + 1], FP8, name="v_sb")
    vt_sb = p_qkv.tile([PT, H, HD + 1], FP8, name="vt_sb")
    # text V duplicated at partition bases 0 and 64 so the av matmul's
    # stationary base matches the packed ppt moving operand
    vt2 = p_qkv.tile([128, H, HD + 1], FP8, name="vt2")

    p_wblk = tc.alloc_tile_pool(name="p_wblk", bufs=4)
    p_wvh = tc.alloc_tile_pool(name="p_wvh", bufs=2)

    skip_bq = "bq" in triv
    skip_kb = "kb" in triv

    def emit_q(m):
        blk = p_wblk.tile([128, KT, 128], FP8, tag="wblk", name="wqblk")
        nc.sync.dma_start(out=blk, in_=wq_d[m])
        for s in range(2):
            sl = slice(s * HLF, (s + 1) * HLF)
            ps = mm_psum()
            for o in range(KT // 2):
                nc.tensor.matmul(
                    ps, blk[:, 2 * o:2 * o + 2, :],
                    xb8[:, 2 * o:2 * o + 2, sl],
                    start=(o == 0), stop=(o == KT // 2 - 1), perf_mode=DROW)
            tmp = tmps.tile([128, 512], F32, tag="wrk", name="q_tmp")
            nc.vector.scalar_tensor_tensor(
                out=tmp, in0=negm_b[:, sl], scalar=st["wqsum"][:, m:m + 1],
                in1=ps, op0=ALU.mult, op1=ALU.add)
            nc.vector.tensor_tensor(qT[:, m, sl], tmp, s8_b[:, sl], ALU.mult)
            if not skip_bq:
                nc.vector.tensor_scalar_add(
                    out=qT[:, m, sl], in0=qT[:, m, sl],
                    scalar1=st["bq8"][:, m:m + 1])

    def emit_k(m):
        blk = p_wblk.tile([128, KT, 128], FP8, tag="wblk", name="wkblk")
        nc.sync.dma_start(out=blk, in_=wk_d[m])
        for s in range(2):
            sl = slice(s * HLF, (s + 1) * HLF)
            ps = mm_psum()
            for o in range(KT // 2):
                nc.tensor.matmul(
                    ps, blk[:, 2 * o:2 * o + 2, :],
                    xb8[:, 2 * o:2 * o + 2, sl],
                    start=(o == 0), stop=(o == KT // 2 - 1), perf_mode=DROW)
            nc.vector.scalar_tensor_tensor(
                out=kT[:, m, sl], in0=negm_b[:, sl],
                scalar=st["wksum"][:, m:m + 1], in1=ps,
                op0=ALU.mult, op1=ALU.add)
            if not skip_kb:
                nc.vector.tensor_scalar_add(
                    out=kT[:, m, sl], in0=kT[:, m, sl],
                    scalar1=st["kb"][:, m:m + 1])
        ps = mm_psum(PT)
        for o in range(KT // 2):
            nc.tensor.matmul(
                ps, blk[:, 2 * o:2 * o + 2, :],
                xtb8[:, 2 * o:2 * o + 2, :],
                start=(o == 0), stop=(o == KT // 2 - 1), perf_mode=DROW)
        nc.vector.scalar_tensor_tensor(
            out=kT[:, m, N:N + PT], in0=negm3_b,
            scalar=st["wksum"][:, m:m + 1], in1=ps,
            op0=ALU.mult, op1=ALU.add)
        if not skip_kb:
            nc.vector.tensor_scalar_add(
                out=kT[:, m, N:N + PT], in0=kT[:, m, N:N + PT],
                scalar1=st["kb"][:, m:m + 1])

    def load_wv(sn):
        wvh = p_wvh.tile([128, KT, 512], BF16, tag="wvh", name="wvh")
        nc.sync.dma_start(out=wvh, in_=wv_d[sn])
        return wvh

    def emit_v(wvh, sn, t):
        ps = mm_psum()
        for o in range(KT):
            nc.tensor.matmul(
                ps, xbT[:, o, t * 128:(t + 1) * 128], wvh[:, o, :],
                start=(o == 0), stop=(o == KT - 1))
        tmp = tmps.tile([128, 512], F32, tag="wrk", name="v_tmp")
        sl = slice(sn * 512, (sn + 1) * 512)
        nc.vector.scalar_tensor_tensor(
            out=tmp, in0=wvsum_b[:, sl], scalar=nm_col[:, t:t + 1],
            in1=ps, op0=ALU.mult, op1=ALU.add)
        out = v_sb[:, t, sn * 8:(sn + 1) * 8, 0:HD]
        nc.vector.tensor_scalar_mul(out=out, in0=tmp,
                                    scalar1=s_col[:, t:t + 1])

    # f32 copy of v_text row 0 (c_vec must not eat the fp8 quantization —
    # its error would bias every token)
    v0_row = rows.tile([1, D], F32, name="v0_row")

    def emit_vt(wvh, sn):
        ps = mm_psum(parts=PT)
        for o in range(KT):
            nc.tensor.matmul(
                ps, xtb[:, o, :], wvh[:, o, :],
                start=(o == 0), stop=(o == KT - 1))
        tmp = tmps.tile([128, 512], F32, tag="wrk", name="vt_tmp")[:PT, :]
        sl = slice(sn * 512, (sn + 1) * 512)
        nc.vector.scalar_tensor_tensor(
            out=tmp, in0=wvsum_b[:PT, sl], scalar=nm3_col,
            in1=ps, op0=ALU.mult, op1=ALU.add)
        out = vt_sb[:, sn * 8:(sn + 1) * 8, 0:HD]
        nc.vector.tensor_scalar_mul(out=out, in0=tmp, scalar1=s3_col)
        nc.vector.tensor_scalar_mul(out=v0_row[:, sl], in0=tmp[0:1, :],
                                    scalar1=s3_col[0:1])

    # pre-attention: only what pair 0's scores need (Q0, K0, Q1, K1) — all
    # V work moves into the first filler slots so the ACT exp stream starts
    # as early as possible
    emit_q(0)
    emit_k(0)
    emit_q(1)
    emit_k(1)
    wvh0 = load_wv(0)
    wvh1 = load_wv(1)

    # ---- c_vec: tanh(gate)*v0_raw + (1+tanh(gate))*bv_eff ---------------
    grws = rows.tile([1, 3, H], F32, name="grws")
    cvw = rows.tile([128, H, HD], F32, name="cvw")
    c_st = consts.tile([128, DT], F32, name="c_st")

    def emit_cvec():
        g_row = grws[:, 0, :]
        th_row = grws[:, 1, :]
        c_work = cvw[0:1]
        nc.sync.dma_start(out=g_row,
                          in_=gate_d.rearrange("(a h) -> a h", a=1))
        nc.scalar.activation(out=th_row, in_=g_row, func=AF.Tanh)
        nc.vector.tensor_copy(
            out=c_work, in_=v0_row.rearrange("a (h d) -> a h d", h=H))
        nc.vector.tensor_tensor(
            c_work, c_work, th_row[:, :, None].to_broadcast((1, H, HD)),
            ALU.mult)
        if "bv" not in triv:
            th1_row = grws[:, 2, :]
            nc.scalar.activation(out=th1_row, in_=th_row, func=AF.Identity,
                                 bias=1.0)
            bv_row = cvw[64:65]
            nc.sync.dma_start(
                out=bv_row,
                in_=pvec["bv_eff"].rearrange("(a h d) -> a h d", a=1, h=H))
            nc.vector.tensor_tensor(
                bv_row, bv_row, th1_row[:, :, None].to_broadcast((1, H, HD)),
                ALU.mult)
            nc.vector.tensor_tensor(c_work, c_work, bv_row, ALU.add)
        c_dram = dram.tile([D], F32, name="c_dram")
        nc.sync.dma_start(
            out=c_dram.rearrange("(a h d) -> a h d", a=1, h=H), in_=c_work)
        nc.sync.dma_start(out=c_st,
                          in_=c_dram.rearrange("(o p) -> p o", p=128))
        if "c_vec" in dbg_d:
            nc.sync.dma_start(out=dbg_d["c_vec"][:], in_=c_dram[:])
        # ones column + zero out reference-key-0 (first text token)
        nc.vector.memset(v_sb[:, :, :, HD:HD + 1], 1.0)
        nc.vector.memset(vt_sb[:, :, HD:HD + 1], 1.0)
        nc.vector.memset(vt_sb[0:1, :, :], 0.0)
        nc.vector.tensor_copy(out=vt2[0:PT], in_=vt_sb)
        nc.vector.tensor_copy(out=vt2[64:64 + PT], in_=vt_sb)

    # =====================================================================
    # Attention: 8 head pairs; filler QKV work interleaved per slot.
    # =====================================================================
    p_OlnT = tc.alloc_tile_pool(name="p_OlnT", bufs=1, side="right")
    o_lnT = p_OlnT.tile([128, DT, NTOK], BF16, name="o_lnT")
    # subln stat accumulators: rows at partitions 0 / 64 of one 4KB tile
    oacc = rows.tile([128, NTOK], F32, name="oacc")
    sumO_row = oacc[0:1, :]
    sqO_row = oacc[64:65, :]
    nc.vector.memset(sumO_row, 0.0)
    nc.vector.memset(sqO_row, 0.0)

    # filler work sits as LATE as dependencies allow: the ACT exp stream is
    # the attention bottleneck and runs ~6us/pair behind the PE, so the PE
    # work should be back-loaded to meet it at the tail
    fillers = [
        lambda: tuple(emit_v(wvh0, 0, t) for t in range(5)),
        lambda: (tuple(emit_v(wvh0, 0, t) for t in range(5, 8)),
                 emit_vt(wvh0, 0), emit_vt(wvh1, 1), emit_cvec(),
                 emit_q(2), emit_k(2)),
        lambda: (emit_q(3), emit_k(3)),
        lambda: (emit_q(4), emit_k(4)),
        lambda: (emit_q(5), emit_k(5),
                 emit_v(wvh1, 1, 0), emit_v(wvh1, 1, 1)),
        lambda: (tuple(emit_v(wvh1, 1, t) for t in range(2, 8)),
                 emit_q(6), emit_k(6)),
        lambda: (emit_q(7), emit_k(7)),
        lambda: None,
    ]

    p_attn = tc.alloc_tile_pool(name="p_attn", bufs=2)

    def emit_scores(j):
        pp = [p_attn.tile([128, KT, NTOK], FP8, tag="pp", bufs=4,
                          name=f"pp{hh}") for hh in range(2)]
        ppt = p_attn.tile([128, NTOK], FP8, tag="ppt", bufs=2, name="ppt")
        for kt in range(KT):
            ps2 = [ps_sc.tile([128, 1024], F32, tag="sc",
                              name=f"ps2{hh}") for hh in range(2)]
            for s in range(2):
                for hh in range(2):
                    base = hh * 64
                    nc.tensor.matmul(
                        ps2[hh][:, s * 512:(s + 1) * 512],
                        kT[base:base + 64, j, kt * 128:(kt + 1) * 128],
                        qT[base:base + 64, j, s * 512:(s + 1) * 512],
                        start=True, stop=True, tile_position=(base, 0),
                        skip_group_check=True)
            for hh in range(2):
                nc.scalar.activation(
                    out=pp[hh][:, kt, :], in_=ps2[hh],
                    func=AF.Exp, scale=s16_col[:, kt:kt + 1])
        # text keys: both heads packed into one [128, NTOK] psum + one exp
        ps2t = ps_sc.tile([128, 1024], F32, tag="sc", name="ps2t")
        for s in range(2):
            for hh in range(2):
                base = hh * 64
                nc.tensor.matmul(
                    ps2t[base:base + PT, s * 512:(s + 1) * 512],
                    kT[base:base + 64, j, N:N + PT],
                    qT[base:base + 64, j, s * 512:(s + 1) * 512],
                    start=True, stop=True, tile_position=(base, base),
                    skip_group_check=True)
        nc.scalar.activation(out=ppt, in_=ps2t, func=AF.Exp, scale=s3_col2)
        return pp, ppt

    def emit_av(j, pp, ppt, fast_rb=False):
        se_pr = p_attn.tile([128, NTOK], F32, tag="se_pr", name="se_pr",
                            bufs=1)
        se_r = p_attn.tile([128, NTOK], F32, tag="se_r", name="se_r",
                           bufs=1)
        for hh in range(2):
            h = 2 * j + hh
            base = hh * 64
            for s in range(2):
                ps = mm_psum()[:HD + 1, :]
                for kp in range(KT // 2):
                    nc.tensor.matmul(
                        ps, v_sb[:, 2 * kp:2 * kp + 2, h, :],
                        pp[hh][:, 2 * kp:2 * kp + 2,
                               s * 512:(s + 1) * 512],
                        start=(kp == 0), stop=False, perf_mode=DROW,
                        skip_group_check=True)
                nc.tensor.matmul(
                    ps, vt2[base:base + PT, h, :],
                    ppt[base:base + PT, s * 512:(s + 1) * 512],
                    start=False, stop=True, tile_position=(base, 0),
                    skip_group_check=True)
                nc.vector.tensor_copy(
                    out=o_lnT[base:base + 64, j, s * 512:(s + 1) * 512],
                    in_=ps[0:HD, :])
                nc.vector.tensor_copy(
                    out=se_pr[hh * 64:hh * 64 + 1, s * 512:(s + 1) * 512],
                    in_=ps[HD:HD + 1, :])
        # one call spanning partitions 0..64: the custom-DVE lowering
        # mishandles a nonzero partition offset (row 64 alone comes out
        # garbage), but a 65-partition AP starting at 0 is fine; the
        # unwritten partitions 1..63 are never read.
        nc.vector.reciprocal_approx_fast(out=se_r[0:65, :],
                                         in_=se_pr[0:65, :])
        if j == 0 and "se0" in dbg_d:
            nc.sync.dma_start(out=dbg_d["se0"][0:1, :], in_=se_pr[0:1, :])
            nc.sync.dma_start(out=dbg_d["se0"][1:2, :], in_=se_pr[64:65, :])
        if fast_rb:
            # broadcast 1/se across partitions on the PE (no DRAM round
            # trip) — used for the last pairs where the bounce latency
            # would stall the proj behind the attention tail
            se_rb = p_attn.tile([128, NTOK], BF16, tag="se_rb",
                                name="se_rb", bufs=2)
            nc.vector.tensor_copy(out=se_rb[0:65, :], in_=se_r[0:65, :])
            rbs = []
            for s in range(2):
                sl = slice(s * 512, (s + 1) * 512)
                pr = mm_psum()
                nc.tensor.matmul(pr[0:64, :], ones2[0:1, :], se_rb[0:1, sl],
                                 start=True, stop=True, tile_position=(0, 0),
                                 skip_group_check=True)
                nc.tensor.matmul(pr[64:128, :], ones2[64:65, :],
                                 se_rb[64:65, sl],
                                 start=True, stop=True,
                                 tile_position=(64, 64),
                                 skip_group_check=True)
                rbs.append(pr)
            return tuple(rbs)
        rb = p_attn.tile([128, NTOK], F32, tag="rb", name="rb", bufs=2)
        seb = dram.tile([2, NTOK], F32, tag="seb", bufs=3, name="seb")
        nc.sync.dma_start(out=seb[0:1, :], in_=se_r[0:1, :])
        nc.sync.dma_start(out=seb[1:2, :], in_=se_r[64:65, :])
        nc.sync.dma_start(
            out=rb[0:64, :], in_=seb[0:1, :].to_broadcast((64, NTOK)))
        nc.sync.dma_start(
            out=rb[64:128, :], in_=seb[1:2, :].to_broadcast((64, NTOK)))
        return rb

    def emit_normalize(j, rb):
        oj = o_lnT[:, j, :]
        if isinstance(rb, tuple):
            for s in range(2):
                sl = slice(s * 512, (s + 1) * 512)
                nc.vector.tensor_tensor(oj[:, sl], oj[:, sl], rb[s],
                                        ALU.mult)
        else:
            if j == 0 and "rb0" in dbg_d:
                nc.sync.dma_start(out=dbg_d["rb0"][:], in_=rb[:])
            nc.vector.tensor_tensor(oj, oj, rb, ALU.mult)
        nc.vector.tensor_scalar_add(out=oj, in0=oj,
                                    scalar1=c_st[:, j:j + 1])
        sq_t4 = sqp.tile([128, NTOK], BF16, tag="sq_t", name="sq_t4")
        nc.vector.tensor_tensor(sq_t4, oj, oj, ALU.mult)
        for s in range(2):
            ps_a = mm_psum()[:1, :]
            nc.tensor.matmul(
                ps_a, ones_b, oj[:, s * 512:(s + 1) * 512],
                start=True, stop=True, skip_group_check=True)
            nc.vector.tensor_tensor(
                sumO_row[:, s * 512:(s + 1) * 512],
                sumO_row[:, s * 512:(s + 1) * 512], ps_a, ALU.add)
            ps_b = mm_psum()[:1, :]
            nc.tensor.matmul(
                ps_b, ones_b, sq_t4[:, s * 512:(s + 1) * 512],
                start=True, stop=True, skip_group_check=True)
            nc.vector.tensor_tensor(
                sqO_row[:, s * 512:(s + 1) * 512],
                sqO_row[:, s * 512:(s + 1) * 512], ps_b, ALU.add)

    pend_av = None
    pend_norm = None
    for j in range(8):
        pp, ppt = emit_scores(j)
        fillers[j]()
        if pend_av is not None:
            ja, ppa, ppta = pend_av
            fast = ja >= 6
            rb = emit_av(ja, ppa, ppta, fast_rb=fast)
            if pend_norm is not None:
                emit_normalize(*pend_norm)
                pend_norm = None
            if fast:
                # rb lives in psum: normalize before the pool recycles it
                emit_normalize(ja, rb)
            else:
                pend_norm = (ja, rb)
        pend_av = (j, pp, ppt)
    ja, ppa, ppta = pend_av
    rb = emit_av(ja, ppa, ppta, fast_rb=True)
    if pend_norm is not None:
        emit_normalize(*pend_norm)
    emit_normalize(ja, rb)

    if "qT" in dbg_d:
        nc.sync.dma_start(out=dbg_d["qT"][:], in_=qT[:])
    if "kT" in dbg_d:
        nc.sync.dma_start(out=dbg_d["kT"][:], in_=kT[:])
    if "v" in dbg_d:
        nc.sync.dma_start(out=dbg_d["v"][:], in_=v_sb[:])
    if "v_text" in dbg_d:
        nc.sync.dma_start(out=dbg_d["v_text"][:], in_=vt_sb[:])
    if "o_lnT" in dbg_d:
        nc.sync.dma_start(out=dbg_d["o_lnT"][:], in_=o_lnT[:])

    # free attention-only SBUF (LIFO), then bring in the late pools
    p_attn.release()
    p_wvh.release()
    p_wblk.release()
    p_qkv.release()
    colp.release()
    ebc.release()
    p_xtb.release()

    # =====================================================================
    # subln rows (deferred into proj): ssub_b, nsub_b [128, NTOK]
    # =====================================================================
    lbc = tc.alloc_tile_pool(name="lbc", bufs=1)
    ssub_b = lbc.tile([128, NTOK], F32, name="ssub_b")
    nsub_b = lbc.tile([128, NTOK], F32, name="nsub_b")
    m2_b = lbc.tile([128, NTOK], F32, name="m2_b")
    s2_b = lbc.tile([128, NTOK], F32, name="s2_b")
    sffn_b = lbc.tile([128, NTOK], F32, name="sffn_b")
    nffn_b = lbc.tile([128, NTOK], F32, name="nffn_b")

    sub_bin = dram.tile([2, NTOK], F32, name="sub_bin")
    nc.sync.dma_start(out=sub_bin[0:1, :], in_=sumO_row)
    nc.sync.dma_start(out=sub_bin[1:2, :], in_=sqO_row)
    bout_s = ln_math(sub_bin, NTOK, D, "subln")
    nc.sync.dma_start(out=ssub_b,
                      in_=bout_s[2:3, :].to_broadcast((128, NTOK)))
    nc.sync.dma_start(out=nsub_b,
                      in_=bout_s[4:5, :].to_broadcast((128, NTOK)))

    # =====================================================================
    # proj + residual -> xnbT (bf16); LN2 split by token halves.
    # =====================================================================
    p_xnbT = tc.alloc_tile_pool(name="p_xnbT", bufs=1)
    xnbT = p_xnbT.tile([128, DT, NTOK], BF16, name="xnbT")
    p_x2T = tc.alloc_tile_pool(name="p_x2T", bufs=1)
    x2T = p_x2T.tile([128, DT, NTOK], BF16, name="x2T")
    p_wp = tc.alloc_tile_pool(name="p_wp", bufs=1)
    wp_sb = p_wp.tile([128, KT, D], BF16, name="wp_sb")
    nc.sync.dma_start(out=wp_sb, in_=wp_d.rearrange("k p d -> p k d"))
    skip_bp = "bp" in triv

    def emit_proj_half(s):
        sl = slice(s * HLF, (s + 1) * HLF)
        for m in range(DT):
            ps = mm_psum()
            for o in range(KT):
                nc.tensor.matmul(
                    ps, wp_sb[:, o, m * 128:(m + 1) * 128], o_lnT[:, o, sl],
                    start=(o == 0), stop=(o == KT - 1))
            t1 = tmps.tile([128, 512], F32, tag="wrk", name="pj_t1")
            nc.vector.tensor_tensor(t1, ps, ssub_b[:, sl], ALU.mult)
            nc.vector.scalar_tensor_tensor(
                out=t1, in0=nsub_b[:, sl], scalar=st["wpsum"][:, m:m + 1],
                in1=t1, op0=ALU.mult, op1=ALU.add)
            if not skip_bp:
                nc.vector.tensor_scalar_add(
                    out=t1, in0=t1, scalar1=st["bp"][:, m:m + 1])
            nc.vector.tensor_tensor(xnbT[:, m, sl], t1, xbT[:, m, sl],
                                    ALU.add)

    def emit_ln2_half(s):
        sl = slice(s * HLF, (s + 1) * HLF)
        t = ps_sc.tile([128, 1024], F32, tag="sc", name="ln2ps")
        ps_a, ps_b = t[:1, 0:512], t[:1, 512:1024]
        for o in range(DT):
            sq_t = sqp.tile([128, NTOK], BF16, tag="sq_t",
                            name="sq_t2")[:, :HLF]
            nc.vector.tensor_tensor(sq_t, xnbT[:, o, sl], xnbT[:, o, sl],
                                    ALU.mult)
            nc.tensor.matmul(ps_a, ones_b, xnbT[:, o, sl],
                             start=(o == 0), stop=(o == DT - 1),
                             skip_group_check=True)
            nc.tensor.matmul(ps_b, ones_b, sq_t,
                             start=(o == 0), stop=(o == DT - 1),
                             skip_group_check=True)
        bout2 = ln_chain(ps_a, ps_b, HLF, D, f"ln2_{s}")
        nc.sync.dma_start(out=m2_b[:, sl],
                          in_=bout2[0:1, :HLF].to_broadcast((128, HLF)))
        nc.sync.dma_start(out=s2_b[:, sl],
                          in_=bout2[2:3, :HLF].to_broadcast((128, HLF)))
        for o in range(DT):
            tA = tmps.tile([128, 512], F32, tag="wrk2", name="ln2_tA",
                           bufs=2)
            nc.vector.tensor_tensor(tA, xnbT[:, o, sl], m2_b[:, sl],
                                    ALU.subtract)
            nc.vector.tensor_tensor(x2T[:, o, sl], tA, s2_b[:, sl],
                                    ALU.mult)

    emit_proj_half(0)
    emit_ln2_half(0)
    emit_proj_half(1)
    emit_ln2_half(1)
    p_wp.release()
    if "xnbT" in dbg_d:
        nc.sync.dma_start(out=dbg_d["xnbT"][:], in_=xnbT[:])
    if "x2T" in dbg_d:
        nc.sync.dma_start(out=dbg_d["x2T"][:], in_=x2T[:])

    # =====================================================================
    # fc1 + gelu -> hT; ffn stats (hierarchical bf16 accumulation on DVE);
    # ffn rows split by halves, deferred into fc2.
    # =====================================================================
    p_hT = tc.alloc_tile_pool(name="p_hT", bufs=1, side="right")
    hT = p_hT.tile([128, HT, NTOK], BF16, name="hT")

    skip_b1 = "b1" in triv

    p_facc = tc.alloc_tile_pool(name="p_facc", bufs=1)
    facc_s = p_facc.tile([128, NTOK], F32, name="facc_s")
    facc_q = p_facc.tile([128, NTOK], F32, name="facc_q")
    f16_s = p_facc.tile([128, NTOK], BF16, name="f16_s")
    f16_q = p_facc.tile([128, NTOK], BF16, name="f16_q")
    p_w1blk = tc.alloc_tile_pool(name="p_w1blk", bufs=3)

    nc.vector.memset(facc_s, 0.0)
    nc.vector.memset(facc_q, 0.0)
    nc.vector.memset(f16_s, 0.0)
    nc.vector.memset(f16_q, 0.0)
    # the first eight hm blocks run s=0 only, so the LN2(s1) row chain can
    # finish behind the s=0 matmuls without idling the PE
    fc1_order = [(hm, 0) for hm in range(8)] \
        + [(hm, 1) for hm in range(8)] \
        + [(hm, s) for hm in range(8, HT) for s in range(2)]
    last_hm = -1
    w1blk = None
    cnt = {0: 0, 1: 0}
    for hm, s in fc1_order:
        if hm != last_hm:
            w1blk = p_w1blk.tile([128, KT, 128], BF16, tag="w1blk",
                                 name="w1blk")
            nc.sync.dma_start(out=w1blk, in_=w1_d[hm])
            last_hm = hm
        sl = slice(s * HLF, (s + 1) * HLF)
        ps = mm_psum()
        for o in range(KT):
            nc.tensor.matmul(
                ps, w1blk[:, o, :], x2T[:, o, sl],
                start=(o == 0), stop=(o == KT - 1))
        hslice = hT[:, hm, sl]
        if skip_b1:
            nc.scalar.activation(out=hslice, in_=ps, func=AF.Gelu)
        else:
            nc.scalar.activation(
                out=hslice, in_=ps, func=AF.Gelu,
                bias=st["b1"][:, hm:hm + 1], scale=1.0)
        sq_t = sqp.tile([128, NTOK], BF16, tag="sq_t",
                        name="sq_tf")[:, :HLF]
        nc.vector.tensor_tensor(sq_t, hslice, hslice, ALU.mult)
        nc.vector.tensor_tensor(f16_s[:, sl], f16_s[:, sl], hslice,
                                ALU.add)
        nc.vector.tensor_tensor(f16_q[:, sl], f16_q[:, sl], sq_t,
                                ALU.add)
        cnt[s] += 1
        if cnt[s] % 8 == 0:
            nc.vector.tensor_tensor(facc_s[:, sl], facc_s[:, sl],
                                    f16_s[:, sl], ALU.add)
            nc.vector.tensor_tensor(facc_q[:, sl], facc_q[:, sl],
                                    f16_q[:, sl], ALU.add)
            nc.vector.memset(f16_s[:, sl], 0.0)
            nc.vector.memset(f16_q[:, sl], 0.0)

    def emit_ffn_rows(s):
        sl = slice(s * HLF, (s + 1) * HLF)
        cst_s = sqp.tile([128, NTOK], BF16, tag="cst", name="cst_s")[:, :HLF]
        cst_q = sqp.tile([128, NTOK], BF16, tag="sq_t", name="cst_q")[:, :HLF]
        nc.scalar.copy(out=cst_s, in_=facc_s[:, sl])
        nc.scalar.copy(out=cst_q, in_=facc_q[:, sl])
        t = ps_sc.tile([128, 1024], F32, tag="sc", name="ffnps")
        ps_a, ps_b = t[:1, 0:512], t[:1, 512:1024]
        nc.tensor.matmul(ps_a, ones_b, cst_s, start=True, stop=True,
                         skip_group_check=True)
        nc.tensor.matmul(ps_b, ones_b, cst_q, start=True, stop=True,
                         skip_group_check=True)
        boutf = ln_chain(ps_a, ps_b, HLF, HID, f"ffn_{s}")
        nc.sync.dma_start(out=sffn_b[:, sl],
                          in_=boutf[2:3, :HLF].to_broadcast((128, HLF)))
        nc.sync.dma_start(out=nffn_b[:, sl],
                          in_=boutf[4:5, :HLF].to_broadcast((128, HLF)))

    emit_ffn_rows(0)
    emit_ffn_rows(1)
    p_w1blk.release()
    p_facc.release()
    p_x2T.release()
    if "hT" in dbg_d:
        nc.sync.dma_start(out=dbg_d["hT"][:], in_=hT[:])

    # =====================================================================
    # fc2 (deferred ffn_ln) + residual + transpose + store
    # =====================================================================
    skip_b2 = "b2" in triv
    with tc.tile_pool(name="p_w2blk", bufs=3) as p_w2blk, \
         tc.tile_pool(name="p_out", bufs=2) as p_out, \
         tc.tile_pool(name="p_stg2", bufs=4) as p_stg2:
        def store_out(outm, m, s):
            for t in range(4):
                pst = mm_psum(128)
                nc.tensor.transpose(
                    pst, outm[:, t * 128:(t + 1) * 128], ident)
                stg = p_stg2.tile([128, 128], F32, tag="stg", name="stg2")
                nc.scalar.copy(out=stg, in_=pst)
                nc.sync.dma_start(
                    out=y_d[(s * 4 + t) * 128:(s * 4 + t + 1) * 128,
                            m * 128:(m + 1) * 128],
                    in_=stg)

        pend_out = None  # transpose/store lags one group so the PE never
        for m in range(DT):  # waits on the DVE epilogue of the current one
            w2blk = p_w2blk.tile([128, HT, 128], BF16, tag="w2blk",
                                 name="w2blk")
            nc.sync.dma_start(out=w2blk, in_=w2_d[m])
            for s in range(2):
                sl = slice(s * HLF, (s + 1) * HLF)
                outm = p_out.tile([128, 512], F32, tag="outm", name="outm")
                ps = mm_psum()
                for o in range(HT):
                    nc.tensor.matmul(
                        ps, w2blk[:, o, :], hT[:, o, sl],
                        start=(o == 0), stop=(o == HT - 1))
                t1 = tmps.tile([128, 512], F32, tag="wrk", name="f2_t1")
                nc.vector.tensor_tensor(t1, ps, sffn_b[:, sl], ALU.mult)
                nc.vector.scalar_tensor_tensor(
                    out=t1, in0=nffn_b[:, sl], scalar=st["w2sum"][:, m:m + 1],
                    in1=t1, op0=ALU.mult, op1=ALU.add)
                if not skip_b2:
                    nc.vector.tensor_scalar_add(
                        out=t1, in0=t1, scalar1=st["b2"][:, m:m + 1])
                nc.vector.tensor_tensor(outm, t1, xnbT[:, m, sl], ALU.add)
                if pend_out is not None:
                    store_out(*pend_out)
                pend_out = (outm, m, s)
        store_out(*pend_out)
    p_hT.release()
    p_OlnT.release()
    p_xnbT.release()
    lbc.release()
    p_xbT.release()
    ctx.close()


# --------------------------------------------------------------------------
# host glue
# --------------------------------------------------------------------------

_PROGRAM_CACHE = {}


def get_program(debug=(), triv=frozenset()):
    key = (tuple(sorted(debug)), tuple(sorted(triv)))
    if key not in _PROGRAM_CACHE:
        _PROGRAM_CACHE[key] = build_program(debug=key[0], triv=key[1])
    return _PROGRAM_CACHE[key]


def _host_fold(inputs):
    """Fold LN gains/biases into weights; compute effective biases and the
    column sums used by the deferred-LN epilogues."""
    f32 = np.float32

    def host(name):
        return np.asarray(inputs[name], dtype=f32)

    g1 = host("n1_g")
    b1n = host("n1_b")
    g3 = host("n3_g")
    b3n = host("n3_b")
    if not (np.array_equal(g1, g3) and np.array_equal(b1n, b3n)):
        assert np.all(g1 == 1.0) and np.all(b1n == 0.0) \
            and np.all(g3 == 1.0) and np.all(b3n == 0.0), \
            "distinct non-trivial n1/n3 gains unsupported"
    wq_eff = g1[:, None] * host("Wq")
    wk_eff = g1[:, None] * host("Wk")
    wv_eff = g1[:, None] * host("Wv")
    bq_eff = host("bq") + b1n @ host("Wq")
    kb_eff = b1n @ host("Wk")
    bv_eff = host("bv") + b1n @ host("Wv")
    wp_eff = host("ln_g")[:, None] * host("Wp")
    bp_eff = host("bp") + host("ln_b") @ host("Wp")
    g2 = host("n2_g")
    b2n = host("n2_b")
    w1_eff = g2[:, None] * host("W1")
    b1_eff = host("b1") + b2n @ host("W1")
    w2_eff = host("ffn_g")[:, None] * host("W2")
    b2_eff = host("b2") + host("ffn_b") @ host("W2")
    return dict(wq_eff=wq_eff, wk_eff=wk_eff, wv_eff=wv_eff,
                bq_eff=bq_eff, kb_eff=kb_eff, bv_eff=bv_eff,
                wp_eff=wp_eff, bp_eff=bp_eff,
                w1_eff=w1_eff, b1_eff=b1_eff,
                w2_eff=w2_eff, b2_eff=b2_eff)


def compute_triv(inputs):
    e = _host_fold(inputs)
    triv = set()
    for k, v in [("bq", e["bq_eff"]), ("kb", e["kb_eff"]),
                 ("bv", e["bv_eff"]), ("bp", e["bp_eff"]),
                 ("b1", e["b1_eff"]), ("b2", e["b2_eff"])]:
        if np.all(v == 0.0):
            triv.add(k)
    return frozenset(triv)


def make_in_maps(inputs):
    bf = ml_dtypes.bfloat16
    f32 = np.float32
    e = _host_fold(inputs)

    fp8 = ml_dtypes.float8_e4m3
    wq_f8 = (e["wq_eff"] * 16.0).astype(fp8)
    wk_f8 = (e["wk_eff"] * 16.0).astype(fp8)
    wv_bf = e["wv_eff"].astype(bf)
    wp_bf = e["wp_eff"].astype(bf)
    w1_bf = e["w1_eff"].astype(bf)
    w2_bf = e["w2_eff"].astype(bf)
    shared = dict(
        wq=np.ascontiguousarray(
            wq_f8.reshape(KT, 128, DT, 128).transpose(2, 1, 0, 3)),
        wk=np.ascontiguousarray(
            wk_f8.reshape(KT, 128, DT, 128).transpose(2, 1, 0, 3)),
        wv=np.ascontiguousarray(
            wv_bf.reshape(KT, 128, 2, 512).transpose(2, 1, 0, 3)),
        wp=wp_bf.reshape(KT, 128, D),
        w1=np.ascontiguousarray(
            w1_bf.reshape(KT, 128, HT, 128).transpose(2, 1, 0, 3)),
        w2=np.ascontiguousarray(
            w2_bf.reshape(HT, 128, DT, 128).transpose(2, 1, 0, 3)),
        wqsum=wq_f8.astype(f32).sum(0),
        wksum=wk_f8.astype(f32).sum(0),
        wvsum=wv_bf.astype(f32).sum(0),
        wpsum=wp_bf.astype(f32).sum(0),
        w2sum=w2_bf.astype(f32).sum(0),
        bq8=(e["bq_eff"] * 0.125).astype(f32),
        kb=(e["kb_eff"] * 16.0).astype(f32),
        bv_eff=e["bv_eff"].astype(f32),
        bp=e["bp_eff"].astype(f32),
        b1=e["b1_eff"].astype(f32),
        b2=e["b2_eff"].astype(f32),
        ident=np.eye(128, dtype=f32),
        gate=np.asarray(inputs["gate"], f32).reshape(H),
    )
    x = np.asarray(inputs["x"], f32)
    xt = np.asarray(inputs["x_text"], f32)
    in_maps = []
    for b in range(B):
        m = dict(shared)
        m["x"] = np.ascontiguousarray(x[b])
        m["x_text"] = np.ascontiguousarray(xt[b])
        in_maps.append(m)
    return in_maps


def kernel(**inputs) -> np.ndarray:
    from concourse.bass_utils import run_bass_kernel_spmd

    nc = get_program(triv=compute_triv(inputs))
    in_maps = make_in_maps(inputs)
    res = run_bass_kernel_spmd(nc, in_maps, list(range(B)))
    out = np.stack([np.asarray(res.results[b]["y"]) for b in range(B)], axis=0)
    return out.astype(np.float32)
